# revision 78
# baseline (speedup 1.0000x reference)
"""Multi-head self-attention (B=4, S=2048, D=1024, H=8) on 8 TRN2 NeuronCores.

Sharding: core c -> batch b=c//2, head-group g=c%2 (4 heads/core).
Each core computes its 4 heads' attention output [512, 2048] (transposed,
head-major); the host gathers/reassembles the full [B, S, D] output.

Notes on the math: the reference adds the source mask per-QUERY (constant
along the key axis) before a softmax over keys, so the mask cancels exactly;
encoder_output_embedding and the target mask are unused by the reference.
The kernel therefore computes pure softmax(q k^T / sqrt(dh)) v, with the
1/sqrt(dh) scale folded into w_query on the host.

Schedule (per core):
  A) V = x @ wv, d-outer over 8 PSUM banks so the first matmul only waits
     on ~2 DMA chunks and the d-loop streams behind the DMA queue (the
     HWDGE processes one descriptor set per ~625ns, so inputs arrive as
     28 x 256KB chunks, not 56 x 128KB).
  B) head 0 q/k projection (PE-serial; nothing to hide it under).
  C) per head: flash-style attention with the next head's projection
     matmuls drip-fed into the ACT-paced inner loop. ACT (exp) has slack
     in heads 0-2 but is the binding engine in head 3, so head 2 hosts
     head 3's full projection in its first block and pre-executes the
     first 8 QK+exp steps of head 3's first block in its second; head 3's
     first block pre-executes 3 exp steps of the second. Row sums fold on
     DVE right after each exp; the softmax denominator pipeline overlaps
     the trailing PV matmuls and the next block's QK.
"""

import math
from contextlib import ExitStack

import numpy as np

import concourse.bacc as bacc
import concourse.tile as tile
from concourse import masks, mybir
from concourse.bass_utils import run_bass_kernel_spmd

N_CORES = 8
B, S, D, H = 4, 2048, 1024, 8
DH = 128                    # head dim
HPC = 4                     # heads per core
DHG = HPC * DH              # 512: projected width per core
SCALE = 1.0 / math.sqrt(DH)
KT = S // 128               # 16 key tiles
ND = D // 128               # 8 contraction tiles
NSB = S // 512              # 4 column blocks of x

F32 = mybir.dt.float32
F16 = mybir.dt.float16

TRACE = False               # test.py flips this for profiling runs
_CACHE = {}


def _emit(tc, nc, xt_ap, wq_ap, wk_ap, wv_ap, sel_ap, out_ap):
    with ExitStack() as ctx:
        p_xt = ctx.enter_context(tc.tile_pool(name="xt", bufs=16))
        p_w = ctx.enter_context(tc.tile_pool(name="w", bufs=4))
        # 3 live per tag: head h-1 still being read by its last block while
        # head h is read and head h+1 is being projected (plan shifts the
        # projections one block early)
        p_qt = ctx.enter_context(tc.tile_pool(name="qt", bufs=3))
        p_v = ctx.enter_context(tc.tile_pool(name="v", bufs=KT))
        p_exp = ctx.enter_context(tc.tile_pool(name="exp", bufs=6))
        p_out = ctx.enter_context(tc.tile_pool(name="o", bufs=2))
        p_rc = ctx.enter_context(tc.tile_pool(name="rc", bufs=2))
        p_const = ctx.enter_context(tc.tile_pool(name="const", bufs=1))
        p_dram = ctx.enter_context(tc.tile_pool(name="dram", bufs=2, space="DRAM"))

        ones = p_const.tile([128, 1], F16, tag="ones")
        nc.vector.memset(ones[:], 1.0)
        ones_row = p_const.tile([1, 128], F16, tag="ones_row")
        nc.vector.memset(ones_row[:], 1.0)
        ident = p_const.tile([128, 128], F16, tag="ident")
        masks.make_identity(nc, ident[:])
        sel8 = p_const.tile([8, 1024], F16, tag="sel8")

        # DMA chunking: [128, 1024] chunks (2KB/partition) halve the count
        # of 625ns HWDGE descriptor slots vs per-tile loads. xh[d][half]
        # covers x^T rows d*128.. cols half*1024..; wc[name][dp] packs two
        # 128-row weight chunks side by side.
        xh = [[None, None] for _ in range(ND)]
        wc = {"wv": [None] * 4, "wq": [None] * 4, "wk": [None] * 4}

        fine = {}                       # (kind, idx): [128, 512] head tiles

        def xts(d, sb):
            t = fine.get(("x", d, sb))
            if t is not None:
                return t[:]
            return xh[d][sb // 2][:, (sb % 2) * 512:(sb % 2) * 512 + 512]

        def wsl(name, d):
            t = fine.get((name, d))
            if t is not None:
                return t[:]
            return wc[name][d // 2][:, (d % 2) * DHG:(d % 2) * DHG + DHG]

        def dma_x(d, half):
            t = p_xt.tile([128, 1024], F16, tag="xt", name=f"x{d}_{half}")
            nc.sync.dma_start(
                t[:], xt_ap[d * 128:(d + 1) * 128, half * 1024:(half + 1) * 1024]
            )
            xh[d][half] = t

        def dma_w(name, ap, dp):
            # host lays weights out d-major: ap is [128, ND*DHG]
            t = p_w.tile([128, 2 * DHG], F16, tag=name, name=f"{name}{dp}")
            nc.sync.dma_start(t[:], ap[:, dp * 2 * DHG:(dp + 1) * 2 * DHG])
            wc[name][dp] = t

        # the first matmul needs only (wv[0], x[0, sb0]): issue those as
        # [128, 512] singles so PE starts ~1us sooner
        for d0 in range(2):
            t = p_w.tile([128, DHG], F16, tag="wvf", bufs=2, name=f"wvf{d0}")
            nc.sync.dma_start(t[:], wv_ap[:, d0 * DHG:(d0 + 1) * DHG])
            fine[("wv", d0)] = t
            t = p_xt.tile([128, 512], F16, tag="xtf", bufs=2, name=f"xf{d0}")
            nc.sync.dma_start(t[:], xt_ap[d0 * 128:(d0 + 1) * 128, 0:512])
            fine[("x", d0, 0)] = t
        for dp in range(4):
            if dp > 0:
                dma_w("wv", wv_ap, dp)
            dma_x(2 * dp, 0)
            dma_x(2 * dp + 1, 0)
        for dp in range(4):
            dma_w("wq", wq_ap, dp)
            dma_x(2 * dp, 1)
            dma_x(2 * dp + 1, 1)
        for dp in range(4):
            dma_w("wk", wk_ap, dp)
        nc.sync.dma_start(sel8[:], sel_ap[:, :])   # only needed at the tail

        # ---- Phase A: V(heads 0-2) = x @ wv[:, :384], d-outer over PSUM --
        # head 3's V columns are deferred into block 5 (head2-qb1), which
        # otherwise has no projection work to hide its ACT-bound exp loop.
        V012 = 3 * DH
        vts = [None] * KT
        vts3 = [None] * (KT // 2)       # head-3 V, [128, 256] per st-pair
        ps_v = tc.alloc_tile_pool(name="psv", bufs=8, space="PSUM")
        if True:
            for wave in range(2):
                pss = [
                    ps_v.tile([128, 512], F32, tag="v", name=f"psv{wave}_{i}")
                    for i in range(8)
                ]
                for d in range(ND):
                    for i in range(8):
                        st = wave * 8 + i
                        nc.tensor.matmul(
                            pss[i][:, 0:V012],
                            xts(d, st // 4)[:, (st % 4) * 128:(st % 4 + 1) * 128],
                            wsl("wv", d)[:, 0:V012],
                            start=(d == 0),
                            stop=(d == ND - 1),
                        )
                for i in range(8):
                    st = wave * 8 + i
                    vt = p_v.tile([128, V012], F16, tag="v", name=f"vt{st}")
                    # alternate engines so the copies drain in half the time
                    # and wave 2's first matmuls aren't blocked on bank reuse
                    if i % 2 == 0:
                        nc.scalar.activation(
                            vt[:], pss[i][:, 0:V012],
                            mybir.ActivationFunctionType.Copy,
                        )
                    else:
                        nc.vector.tensor_copy(vt[:], pss[i][:, 0:V012])
                    vts[st] = vt

        # ---- Attention-phase PSUM pools (4 + 2 + 2 = 8 banks) ----------
        def proj_steps(h, pool, copy_alt=False, tag="proj"):
            """Create head h's q/k tiles; return (qt, kt, generator) where
            the generator emits one 2-matmul chunk per next()."""
            qt = p_qt.tile([128, S], F16, tag="qt", name=f"qt{h}")
            kt = p_qt.tile([128, S], F16, tag="kt", name=f"kt{h}")

            def gen():
                n = 0
                for dst, wname in ((qt, "wq"), (kt, "wk")):
                    for sb in range(NSB):
                        ps = pool.tile([128, 512], F32, tag=tag,
                                       name=f"pj{h}_{wname}{sb}")
                        for d in range(ND):
                            nc.tensor.matmul(
                                ps[:],
                                wsl(wname, d)[:, h * 128:(h + 1) * 128],
                                xts(d, sb),
                                start=(d == 0),
                                stop=(d == ND - 1),
                            )
                            if d % 2 == 1:
                                yield None
                        dsl = dst[:, sb * 512:(sb + 1) * 512]
                        if copy_alt and n % 2 == 0:
                            nc.scalar.activation(
                                dsl, ps[:], mybir.ActivationFunctionType.Copy
                            )
                        else:
                            nc.vector.tensor_copy(dsl, ps[:])
                        n += 1
                while True:
                    yield None

            return qt, kt, gen()

        # per-block attention state shared between qk/pv/norm emitters
        class Blk:
            def __init__(self, h, qb):
                self.h, self.qb = h, qb
                self.q0 = qb * 1024
                self.ets = {}
                self.acc = None
                self.pv = None
                self.final = False

        def qk_step(blk, qt, kt, k, pre=False):
            st_ps = ps_mm.tile([128, 1024], F32, tag="sT",
                               name=f"sT{blk.h}_{blk.qb}_{k}")
            for hf in range(2):
                nc.tensor.matmul(
                    st_ps[:, hf * 512:(hf + 1) * 512],
                    kt[:, k * 128:(k + 1) * 128],
                    qt[:, blk.q0 + hf * 512:blk.q0 + (hf + 1) * 512],
                    start=True,
                    stop=True,
                )
            et = p_exp.tile([128, 1024], F16, tag="pre" if pre else "exp",
                            bufs=14 if pre else 9, name=f"et{blk.h}_{blk.qb}_{k}")
            nc.scalar.activation(et[:], st_ps[:], mybir.ActivationFunctionType.Exp)
            blk.ets[k] = et
            # fold the row-sum accumulator as soon as exp lands, so the
            # denominator chain starts before the last PV
            if blk.final:
                # two parallel fold chains (even/odd k through 13) so no
                # serial DVE chain lags the tail; et14/et15 are summed by
                # the row-sum matmuls directly
                if k == 2:
                    acc = p_exp.tile([128, 1024], F16, tag="acc", bufs=5,
                                     name="acc_e")
                    nc.vector.tensor_add(acc[:], blk.ets[0][:], et[:])
                    blk.acc = acc
                elif k == 3:
                    acc = p_exp.tile([128, 1024], F16, tag="acc", bufs=5,
                                     name="acc_o")
                    nc.vector.tensor_add(acc[:], blk.ets[1][:], et[:])
                    blk.acc_o = acc
                elif 4 <= k <= 13:
                    dst = blk.acc if k % 2 == 0 else blk.acc_o
                    nc.vector.tensor_add(dst[:], dst[:], et[:])
            elif k == 1:
                acc = p_exp.tile([128, 1024], F16, tag="acc", bufs=5,
                                 name=f"acc{blk.h}_{blk.qb}")
                nc.vector.tensor_add(acc[:], blk.ets[0][:], et[:])
                blk.acc = acc
            elif k > 1:
                nc.vector.tensor_add(blk.acc[:], blk.acc[:], et[:])

        def pv_step(blk, k):
            if blk.final and k >= 14:
                et = blk.ets[k]         # norm_final still needs it
            else:
                et = blk.ets.pop(k)
            if blk.h < 3:
                vsl = vts[k][:, blk.h * 128:(blk.h + 1) * 128]
            else:
                vsl = vts3[k // 2][:, (k % 2) * 128:(k % 2 + 1) * 128]
            for hf in range(2):
                sl = slice(hf * 512, (hf + 1) * 512)
                nc.tensor.matmul(
                    blk.pv[:, sl],
                    vsl,
                    et[:, sl],
                    start=(k == 0),
                    stop=(k == KT - 1),
                )

        def norm_steps(blk, ps_pj):
            """Softmax-denominator chain for a non-final block; yields so the
            caller interleaves it with the next block's emission."""
            h, qb, acc, pv = blk.h, blk.qb, blk.acc, blk.pv
            # free the pv PSUM bank right away — the next block's first PV
            # matmul sits behind this chain in PE queue order
            ob = p_out.tile([128, 1024], F32, tag="o", name=f"ob{h}{qb}")
            nc.vector.tensor_copy(ob[:], pv[:])
            yield None
            sms = []
            for hf in range(2):
                sm = ps_pj.tile([1, 512], F32, tag="proj", name=f"sm{h}{qb}{hf}")
                nc.tensor.matmul(
                    sm[:], ones[:], acc[:, hf * 512:(hf + 1) * 512],
                    start=True, stop=True,
                )
                sms.append(sm)
            yield None
            sm_sb = p_rc.tile([1, 1024], F32, tag="sm_sb")
            for hf in range(2):
                nc.vector.tensor_copy(sm_sb[:, hf * 512:(hf + 1) * 512], sms[hf][:])
            sm2 = p_rc.tile([128, 8], F32, tag="sm2")
            nc.sync.dma_start(sm2[:], sm_sb[:], single_packet=True)
            rc2 = p_rc.tile([128, 8], F16, tag="rc2")
            with nc.allow_low_precision(reason="fp16 softmax denom"):
                nc.vector.reciprocal(rc2[:], sm2[:])
            r2dram = p_dram.tile([1, 1024], F16, tag="r2dram")
            nc.sync.dma_start(
                r2dram[:].rearrange("a (p c) -> (a p) c", p=128), rc2[:],
                single_packet=True,
            )
            rbc = p_rc.tile([128, 1024], F16, tag="rbc")
            nc.sync.dma_start(rbc[:], r2dram[0:1, :].to_broadcast((128, 1024)))
            yield None
            # halves, so a waiting multiply never blocks the DVE queue long
            obh = p_out.tile([128, 1024], F16, tag="oh", name=f"obh{h}{qb}")
            for hf in range(2):
                sl = slice(hf * 512, (hf + 1) * 512)
                nc.vector.tensor_mul(obh[:, sl], ob[:, sl], rbc[:, sl])
                nc.sync.dma_start(
                    out_ap[h * 128:(h + 1) * 128,
                           qb * 1024 + hf * 512:qb * 1024 + (hf + 1) * 512],
                    obh[:, sl],
                )
                yield None

        def norm_final(blk):
            """Tail chain for the very last block: row sums straight into
            [128, 8] via tiny stationary matmuls over the (k<=13) fold plus
            et14/et15 (PE is idle here and the fold chain lags ~2us), one
            unshuffle DMA, then a PE broadcast — minimizes serial DMAs."""
            h, qb, acc, pv = blk.h, blk.qb, blk.acc, blk.pv
            smq = ps_pj.tile([128, 512], F32, tag="proj", name="smq")
            srcs = [acc, blk.acc_o, blk.ets[14], blk.ets[15]]
            for j in range(8):
                # stationary column m is q = j*128 + m: smq[p, j] holds
                # rowsum(q = j*128 + p)
                for si, src in enumerate(srcs):
                    nc.tensor.matmul(
                        smq[:, j:j + 1], src[:, j * 128:(j + 1) * 128], ones[:],
                        start=(si == 0), stop=(si == len(srcs) - 1),
                        skip_group_check=True,
                    )
            # stage pv in SBUF (DVE may read only one PSUM input, and this
            # overlaps the reciprocal chain)
            ob_pv = p_out.tile([128, 1024], F32, tag="o", name="ob_pv")
            nc.vector.tensor_copy(ob_pv[:], pv[:])
            rc2 = p_rc.tile([128, 8], F16, tag="rc2")
            with nc.allow_low_precision(reason="fp16 softmax denom"):
                nc.vector.reciprocal(rc2[:], smq[:, 0:8])
            # stay on-chip: PE transpose + per-block broadcast matmuls skip
            # the ~2.5us of DMA fixed costs an unshuffle round trip takes
            tps = ps_pj.tile([128, 512], F32, tag="proj", name="tps")
            t16 = tps[:].bitcast(F16)
            nc.tensor.transpose(t16[0:8, 0:128], rc2[:], ident[:])
            t_sb = p_rc.tile([8, 128], F16, tag="t_sb")
            nc.vector.tensor_copy(t_sb[:], t16[0:8, 0:128])
            rbc_ps = ps_mm.tile([128, 1024], F32, tag="sT", name="rbc_ps")
            for j in range(8):
                # sel[:, j-block] is the row-j indicator: out = T[j, :] bcast
                nc.tensor.matmul(
                    rbc_ps[:, j * 128:(j + 1) * 128],
                    sel8[:, j * 128:(j + 1) * 128], t_sb[:],
                    start=True, stop=True, skip_group_check=True,
                )
            for hf in range(2):
                sl = slice(hf * 512, (hf + 1) * 512)
                ob = p_out.tile([128, 512], F16, tag="of", name=f"of{hf}")
                nc.vector.tensor_mul(ob[:], ob_pv[:, sl], rbc_ps[:, sl])
                nc.sync.dma_start(
                    out_ap[h * 128:(h + 1) * 128,
                           qb * 1024 + hf * 512:qb * 1024 + (hf + 1) * 512],
                    ob[:],
                )

        # head 0's projections run serially (nothing to hide them under) and
        # share the phase-A PSUM pool, so they don't wait on its release;
        # heads 1..3 project inside earlier blocks' attention loops.
        q0_, k0_, gen = proj_steps(0, ps_v, copy_alt=True, tag="v")
        for _ in range(33):     # 8 chunks x 4 yields + 1: the final copy
            next(gen)           # is only emitted on the next() AFTER the
                                # last chunk's 4th yield
        ps_v.release()
        ps_mm = ctx.enter_context(tc.tile_pool(name="psmm", bufs=2, space="PSUM"))
        ps_pv = ctx.enter_context(tc.tile_pool(name="pspv", bufs=1, space="PSUM"))

        with tc.tile_pool(name="pspj", bufs=2, space="PSUM") as ps_pj:
            qts, kts = {0: q0_}, {0: k0_}
            projs = {}

            def make_proj(h):
                qth, kth, g = proj_steps(h, ps_pj)
                qts[h], kts[h], projs[h] = qth, kth, g

            make_proj(1)
            blocks = [Blk(b // 2, b % 2) for b in range(8)]
            blocks[7].final = True
            # per-block drip-feed plan:
            #   proj[b] = (head whose projection is injected, total pairs)
            #   pre[b]  = how many qk+exp steps of block b+1 to pre-execute
            # Each exp is 1038ns on ACT vs 852ns of matching qk+pv on PE, so
            # blocks with no injected work go ACT-bound. Shifting every
            # projection one block early lets each block pre-execute the
            # next one's first qk+exp steps (the cascade below), and head
            # 3's V projection fills block 5.
            # 33 next()s per generator: the final copy is emitted on the
            # call after the last chunk's 4th yield
            proj_plan = {0: (1, 33), 1: (2, 16), 2: (2, 17), 3: (3, 16),
                         4: (3, 17)}
            pre_plan = {3: 2, 4: 7, 5: 12, 6: 6}
            norm_in = None

            def emit_v3_chunk(sp):
                # block 5 has no projection in flight, so the proj ring is free
                ps = ps_pj.tile([128, 512], F32, tag="proj", name=f"v3_{sp}")
                for j in range(2):
                    st = sp * 2 + j
                    for d in range(ND):
                        nc.tensor.matmul(
                            ps[:, j * 128:(j + 1) * 128],
                            xts(d, st // 4)[:, (st % 4) * 128:(st % 4 + 1) * 128],
                            wsl("wv", d)[:, V012:DHG],
                            start=(d == 0),
                            stop=(d == ND - 1),
                            skip_group_check=True,
                        )
                vt = p_v.tile([128, 256], F16, tag="v3", bufs=8,
                              name=f"vt3_{sp}")
                nc.vector.tensor_copy(vt[:], ps[:, 0:256])
                vts3[sp] = vt

            for b, blk in enumerate(blocks):
                h = blk.h
                qt, kt = qts[h], kts[h]
                blk.pv = ps_pv.tile([128, 1024], F32, tag="pv",
                                    name=f"pv{h}_{blk.qb}")
                k0 = len(blk.ets)
                kq, kp, it = k0, 0, 0
                pre_left = pre_plan.get(b, 0)
                ph, pairs_left = proj_plan.get(b, (None, 0))
                if ph is not None and ph not in projs:
                    make_proj(ph)
                nproj = projs.get(ph)
                v3_left = 8 if b == 5 else 0

                if kq < KT:
                    qk_step(blk, qt, kt, kq)
                    kq += 1
                if norm_in is not None:
                    next(norm_in, None)         # pv-freeing copy
                if kq < KT:
                    qk_step(blk, qt, kt, kq)
                    kq += 1
                if norm_in is not None:
                    next(norm_in, None)         # row-sum matmuls

                while kp < KT:
                    if k0 > 0:
                        # pre-filled block: pv leads so the exp-ring WAR
                        # order stays correct (pv(k) must be emitted before
                        # qk(k + ring) reuses et(k)'s slot)
                        pv_step(blk, kp)
                        kp += 1
                        if kq < KT:
                            qk_step(blk, qt, kt, kq)
                            kq += 1
                    elif kq < KT:
                        qk_step(blk, qt, kt, kq)
                        kq += 1
                        if kq - 2 >= kp:
                            pv_step(blk, kp)
                            kp += 1
                    else:
                        pv_step(blk, kp)
                        kp += 1
                    it += 1
                    if it == 1 and norm_in is not None:
                        next(norm_in, None)     # reciprocal DMA chain
                    if it == 5 and norm_in is not None:
                        # broadcast is in flight by now; the multiplies
                        # won't head-of-line-block the DVE queue for long
                        for _ in norm_in:
                            pass
                        norm_in = None
                    iters_left = max(1, 15 - it)
                    n_inj = min(pairs_left,
                                (pairs_left + iters_left - 1) // iters_left)
                    for _ in range(n_inj):
                        next(nproj)
                        pairs_left -= 1
                    if pre_left > 0 and it >= 2:
                        nblk = blocks[b + 1]
                        pk = len(nblk.ets)
                        qk_step(nblk, qts[nblk.h], kts[nblk.h], pk, pre=True)
                        pre_left -= 1
                    if v3_left > 0 and it % 2 == 0:
                        emit_v3_chunk(8 - v3_left)
                        v3_left -= 1
                while pairs_left > 0:
                    next(nproj)
                    pairs_left -= 1
                while v3_left > 0:
                    emit_v3_chunk(8 - v3_left)
                    v3_left -= 1
                if b < 7:
                    norm_in = norm_steps(blk, ps_pj)
                else:
                    if norm_in is not None:
                        for _ in norm_in:
                            pass
                        norm_in = None
                    norm_final(blk)


def _build():
    nc = bacc.Bacc(
        "TRN2",
        target_bir_lowering=False,
        debug=False,
        enable_asserts=False,
        num_devices=N_CORES,
    )
    xt_ap = nc.dram_tensor("xt", [D, S], F16, kind="ExternalInput").ap()
    wq_ap = nc.dram_tensor("wq", [128, ND * DHG], F16, kind="ExternalInput").ap()
    wk_ap = nc.dram_tensor("wk", [128, ND * DHG], F16, kind="ExternalInput").ap()
    wv_ap = nc.dram_tensor("wv", [128, ND * DHG], F16, kind="ExternalInput").ap()
    sel_ap = nc.dram_tensor("sel", [8, 1024], F16, kind="ExternalInput").ap()
    out_ap = nc.dram_tensor("out", [DHG, S], F16, kind="ExternalOutput").ap()
    with tile.TileContext(nc) as tc:
        _emit(tc, nc, xt_ap, wq_ap, wk_ap, wv_ap, sel_ap, out_ap)
    nc.compile()
    return nc


def _shard_inputs(inputs):
    x = np.ascontiguousarray(np.asarray(inputs["input_embeddings"], dtype=np.float32))
    wq = np.asarray(inputs["w_query"], dtype=np.float32) * SCALE
    wk = np.asarray(inputs["w_key"], dtype=np.float32)
    wv = np.asarray(inputs["w_value"], dtype=np.float32)

    def gather(w, g):
        # head h occupies the strided cols d = hd*8 + h; regroup head-major,
        # then d-major so each [128, 1024] DMA chunk is a plain slice
        w4 = w.reshape(D, DH, H)[:, :, g * HPC:(g + 1) * HPC]   # (D, hd, hl)
        wg = w4.transpose(0, 2, 1).reshape(ND, 128, DHG)
        return np.ascontiguousarray(
            wg.transpose(1, 0, 2).reshape(128, ND * DHG).astype(np.float16)
        )

    sel = np.kron(np.eye(8), np.ones((1, 128))).astype(np.float16)
    in_maps = []
    for c in range(N_CORES):
        b, g = divmod(c, 2)
        in_maps.append(
            {
                "xt": np.ascontiguousarray(x[b].T.astype(np.float16)),
                "wq": gather(wq, g),
                "wk": gather(wk, g),
                "wv": gather(wv, g),
                "sel": sel,
            }
        )
    return in_maps


def kernel(**inputs):
    nc = _CACHE.get("nc")
    if nc is None:
        nc = _CACHE["nc"] = _build()
    in_maps = _shard_inputs(inputs)
    res = run_bass_kernel_spmd(
        nc, in_maps, core_ids=list(range(N_CORES)), trace=TRACE
    )
    _CACHE["last_result"] = res
    out = np.empty((B, S, DH, H), dtype=np.float32)
    for c in range(N_CORES):
        b, g = divmod(c, 2)
        o = res.results[c]["out"].reshape(HPC, DH, S)            # (hl, hd, s)
        out[b, :, :, g * HPC:(g + 1) * HPC] = o.transpose(2, 1, 0)
    return out.reshape(B, S, D)


# revision 85
# speedup vs baseline: 1.0038x; 1.0038x over previous
"""Multi-head self-attention (B=4, S=2048, D=1024, H=8) on 8 TRN2 NeuronCores.

Sharding: core c -> batch b=c//2, head-group g=c%2 (4 heads/core).
Each core computes its 4 heads' attention output [512, 2048] (transposed,
head-major); the host gathers/reassembles the full [B, S, D] output.

Notes on the math: the reference adds the source mask per-QUERY (constant
along the key axis) before a softmax over keys, so the mask cancels exactly;
encoder_output_embedding and the target mask are unused by the reference.
The kernel therefore computes pure softmax(q k^T / sqrt(dh)) v, with the
1/sqrt(dh) scale folded into w_query on the host.

Schedule (per core):
  A) V = x @ wv, d-outer over 8 PSUM banks so the first matmul only waits
     on ~2 DMA chunks and the d-loop streams behind the DMA queue (the
     HWDGE processes one descriptor set per ~625ns, so inputs arrive as
     28 x 256KB chunks, not 56 x 128KB).
  B) head 0 q/k projection (PE-serial; nothing to hide it under).
  C) per head: flash-style attention with the next head's projection
     matmuls drip-fed into the ACT-paced inner loop. ACT (exp) has slack
     in heads 0-2 but is the binding engine in head 3, so head 2 hosts
     head 3's full projection in its first block and pre-executes the
     first 8 QK+exp steps of head 3's first block in its second; head 3's
     first block pre-executes 3 exp steps of the second. Row sums fold on
     DVE right after each exp; the softmax denominator pipeline overlaps
     the trailing PV matmuls and the next block's QK.
"""

import math
from contextlib import ExitStack

import numpy as np

import concourse.bacc as bacc
import concourse.tile as tile
from concourse import masks, mybir
from concourse.bass_utils import run_bass_kernel_spmd

N_CORES = 8
B, S, D, H = 4, 2048, 1024, 8
DH = 128                    # head dim
HPC = 4                     # heads per core
DHG = HPC * DH              # 512: projected width per core
SCALE = 1.0 / math.sqrt(DH)
KT = S // 128               # 16 key tiles
ND = D // 128               # 8 contraction tiles
NSB = S // 512              # 4 column blocks of x

F32 = mybir.dt.float32
F16 = mybir.dt.float16

TRACE = False               # test.py flips this for profiling runs
_CACHE = {}


def _emit(tc, nc, xt_ap, wq_ap, wk_ap, wv_ap, sel_ap, out_ap):
    with ExitStack() as ctx:
        p_xt = ctx.enter_context(tc.tile_pool(name="xt", bufs=16))
        p_w = ctx.enter_context(tc.tile_pool(name="w", bufs=4))
        # 3 live per tag: head h-1 still being read by its last block while
        # head h is read and head h+1 is being projected (plan shifts the
        # projections one block early)
        p_qt = ctx.enter_context(tc.tile_pool(name="qt", bufs=3))
        p_v = ctx.enter_context(tc.tile_pool(name="v", bufs=KT))
        p_exp = ctx.enter_context(tc.tile_pool(name="exp", bufs=6))
        p_out = ctx.enter_context(tc.tile_pool(name="o", bufs=2))
        p_rc = ctx.enter_context(tc.tile_pool(name="rc", bufs=2))
        p_const = ctx.enter_context(tc.tile_pool(name="const", bufs=1))
        p_dram = ctx.enter_context(tc.tile_pool(name="dram", bufs=2, space="DRAM"))

        ones = p_const.tile([128, 1], F16, tag="ones")
        nc.vector.memset(ones[:], 1.0)
        ones_row = p_const.tile([1, 128], F16, tag="ones_row")
        nc.vector.memset(ones_row[:], 1.0)
        ident = p_const.tile([128, 128], F16, tag="ident")
        masks.make_identity(nc, ident[:])
        sel8 = p_const.tile([8, 1024], F16, tag="sel8")

        # DMA chunking: [128, 1024] chunks (2KB/partition) halve the count
        # of 625ns HWDGE descriptor slots vs per-tile loads. xh[d][half]
        # covers x^T rows d*128.. cols half*1024..; wc[name][dp] packs two
        # 128-row weight chunks side by side.
        xh = [[None, None] for _ in range(ND)]
        wc = {"wv": [None] * 4, "wq": [None] * 4, "wk": [None] * 4}

        fine = {}                       # (kind, idx): [128, 512] head tiles

        def xts(d, sb):
            t = fine.get(("x", d, sb))
            if t is not None:
                return t[:]
            return xh[d][sb // 2][:, (sb % 2) * 512:(sb % 2) * 512 + 512]

        def wsl(name, d):
            t = fine.get((name, d))
            if t is not None:
                return t[:]
            return wc[name][d // 2][:, (d % 2) * DHG:(d % 2) * DHG + DHG]

        def dma_x(d, half):
            t = p_xt.tile([128, 1024], F16, tag="xt", name=f"x{d}_{half}")
            nc.sync.dma_start(
                t[:], xt_ap[d * 128:(d + 1) * 128, half * 1024:(half + 1) * 1024]
            )
            xh[d][half] = t

        def dma_w(name, ap, dp):
            # host lays weights out d-major: ap is [128, ND*DHG]
            t = p_w.tile([128, 2 * DHG], F16, tag=name, name=f"{name}{dp}")
            nc.sync.dma_start(t[:], ap[:, dp * 2 * DHG:(dp + 1) * 2 * DHG])
            wc[name][dp] = t

        # the first matmul needs only (wv[0], x[0, sb0]): issue those as
        # [128, 512] singles so PE starts ~1us sooner
        for d0 in range(2):
            t = p_w.tile([128, DHG], F16, tag="wvf", bufs=2, name=f"wvf{d0}")
            nc.sync.dma_start(t[:], wv_ap[:, d0 * DHG:(d0 + 1) * DHG])
            fine[("wv", d0)] = t
            t = p_xt.tile([128, 512], F16, tag="xtf", bufs=2, name=f"xf{d0}")
            nc.sync.dma_start(t[:], xt_ap[d0 * 128:(d0 + 1) * 128, 0:512])
            fine[("x", d0, 0)] = t
        for dp in range(4):
            if dp > 0:
                dma_w("wv", wv_ap, dp)
            dma_x(2 * dp, 0)
            dma_x(2 * dp + 1, 0)
        for dp in range(4):
            dma_w("wq", wq_ap, dp)
            dma_x(2 * dp, 1)
            dma_x(2 * dp + 1, 1)
        for dp in range(4):
            dma_w("wk", wk_ap, dp)
        nc.sync.dma_start(sel8[:], sel_ap[:, :])   # only needed at the tail

        # ---- Phase A: V(heads 0-2) = x @ wv[:, :384], d-outer over PSUM --
        # head 3's V columns are deferred into block 5 (head2-qb1), which
        # otherwise has no projection work to hide its ACT-bound exp loop.
        V012 = 3 * DH
        vts = [None] * KT
        vts3 = [None] * (KT // 2)       # head-3 V, [128, 256] per st-pair
        ps_v = tc.alloc_tile_pool(name="psv", bufs=8, space="PSUM")
        if True:
            for wave in range(2):
                pss = [
                    ps_v.tile([128, 512], F32, tag="v", name=f"psv{wave}_{i}")
                    for i in range(8)
                ]
                for d in range(ND):
                    for i in range(8):
                        st = wave * 8 + i
                        nc.tensor.matmul(
                            pss[i][:, 0:V012],
                            xts(d, st // 4)[:, (st % 4) * 128:(st % 4 + 1) * 128],
                            wsl("wv", d)[:, 0:V012],
                            start=(d == 0),
                            stop=(d == ND - 1),
                        )
                for i in range(8):
                    st = wave * 8 + i
                    vt = p_v.tile([128, V012], F16, tag="v", name=f"vt{st}")
                    # alternate engines so the copies drain in half the time
                    # and wave 2's first matmuls aren't blocked on bank reuse
                    if i % 2 == 0:
                        nc.scalar.activation(
                            vt[:], pss[i][:, 0:V012],
                            mybir.ActivationFunctionType.Copy,
                        )
                    else:
                        nc.vector.tensor_copy(vt[:], pss[i][:, 0:V012])
                    vts[st] = vt

        # ---- Attention-phase PSUM pools (4 + 2 + 2 = 8 banks) ----------
        def proj_steps(h, pool, copy_alt=False, tag="proj"):
            """Create head h's q/k tiles; return (qt, kt, generator) where
            the generator emits one 2-matmul chunk per next()."""
            qt = p_qt.tile([128, S], F16, tag="qt", name=f"qt{h}")
            kt = p_qt.tile([128, S], F16, tag="kt", name=f"kt{h}")

            def gen():
                n = 0
                order = [(qt, "wq", 0), (qt, "wq", 1), (kt, "wk", 0),
                         (kt, "wk", 1), (kt, "wk", 2), (kt, "wk", 3),
                         (qt, "wq", 2), (qt, "wq", 3)]
                for dst, wname, sb in order:
                    if True:
                        ps = pool.tile([128, 512], F32, tag=tag,
                                       name=f"pj{h}_{wname}{sb}")
                        for d in range(ND):
                            nc.tensor.matmul(
                                ps[:],
                                wsl(wname, d)[:, h * 128:(h + 1) * 128],
                                xts(d, sb),
                                start=(d == 0),
                                stop=(d == ND - 1),
                            )
                            if d % 2 == 1:
                                yield None
                        dsl = dst[:, sb * 512:(sb + 1) * 512]
                        if copy_alt and n % 2 == 0:
                            nc.scalar.activation(
                                dsl, ps[:], mybir.ActivationFunctionType.Copy
                            )
                        else:
                            nc.vector.tensor_copy(dsl, ps[:])
                        n += 1
                while True:
                    yield None

            return qt, kt, gen()

        # per-block attention state shared between qk/pv/norm emitters
        class Blk:
            def __init__(self, h, qb):
                self.h, self.qb = h, qb
                self.q0 = qb * 1024
                self.ets = {}
                self.acc = None
                self.pv = None
                self.final = False

        def qk_step(blk, qt, kt, k, pre=False):
            st_ps = ps_mm.tile([128, 1024], F32, tag="sT",
                               name=f"sT{blk.h}_{blk.qb}_{k}")
            for hf in range(2):
                nc.tensor.matmul(
                    st_ps[:, hf * 512:(hf + 1) * 512],
                    kt[:, k * 128:(k + 1) * 128],
                    qt[:, blk.q0 + hf * 512:blk.q0 + (hf + 1) * 512],
                    start=True,
                    stop=True,
                )
            et = p_exp.tile([128, 1024], F16, tag="pre" if pre else "exp",
                            bufs=14 if pre else 9, name=f"et{blk.h}_{blk.qb}_{k}")
            nc.scalar.activation(et[:], st_ps[:], mybir.ActivationFunctionType.Exp)
            blk.ets[k] = et
            # fold the row-sum accumulator as soon as exp lands, so the
            # denominator chain starts before the last PV
            if blk.final:
                # two parallel fold chains (even/odd k through 13) so no
                # serial DVE chain lags the tail; et14/et15 are summed by
                # the row-sum matmuls directly
                if k == 2:
                    acc = p_exp.tile([128, 1024], F16, tag="acc", bufs=5,
                                     name="acc_e")
                    nc.vector.tensor_add(acc[:], blk.ets[0][:], et[:])
                    blk.acc = acc
                elif k == 3:
                    acc = p_exp.tile([128, 1024], F16, tag="acc", bufs=5,
                                     name="acc_o")
                    nc.vector.tensor_add(acc[:], blk.ets[1][:], et[:])
                    blk.acc_o = acc
                elif 4 <= k <= 13:
                    dst = blk.acc if k % 2 == 0 else blk.acc_o
                    nc.vector.tensor_add(dst[:], dst[:], et[:])
            elif k == 1:
                acc = p_exp.tile([128, 1024], F16, tag="acc", bufs=5,
                                 name=f"acc{blk.h}_{blk.qb}")
                nc.vector.tensor_add(acc[:], blk.ets[0][:], et[:])
                blk.acc = acc
            elif k > 1:
                nc.vector.tensor_add(blk.acc[:], blk.acc[:], et[:])

        def pv_step(blk, k):
            if blk.final and k >= 14:
                et = blk.ets[k]         # norm_final still needs it
            else:
                et = blk.ets.pop(k)
            if blk.h < 3:
                vsl = vts[k][:, blk.h * 128:(blk.h + 1) * 128]
            else:
                vsl = vts3[k // 2][:, (k % 2) * 128:(k % 2 + 1) * 128]
            for hf in range(2):
                sl = slice(hf * 512, (hf + 1) * 512)
                nc.tensor.matmul(
                    blk.pv[:, sl],
                    vsl,
                    et[:, sl],
                    start=(k == 0),
                    stop=(k == KT - 1),
                )

        def norm_steps(blk, ps_pj):
            """Softmax-denominator chain for a non-final block; yields so the
            caller interleaves it with the next block's emission."""
            h, qb, acc, pv = blk.h, blk.qb, blk.acc, blk.pv
            # free the pv PSUM bank right away — the next block's first PV
            # matmul sits behind this chain in PE queue order
            ob = p_out.tile([128, 1024], F32, tag="o", name=f"ob{h}{qb}")
            nc.vector.tensor_copy(ob[:], pv[:])
            yield None
            sms = []
            for hf in range(2):
                sm = ps_pj.tile([1, 512], F32, tag="proj", name=f"sm{h}{qb}{hf}")
                nc.tensor.matmul(
                    sm[:], ones[:], acc[:, hf * 512:(hf + 1) * 512],
                    start=True, stop=True,
                )
                sms.append(sm)
            yield None
            sm_sb = p_rc.tile([1, 1024], F32, tag="sm_sb")
            for hf in range(2):
                nc.vector.tensor_copy(sm_sb[:, hf * 512:(hf + 1) * 512], sms[hf][:])
            sm2 = p_rc.tile([128, 8], F32, tag="sm2")
            nc.sync.dma_start(sm2[:], sm_sb[:], single_packet=True)
            rc2 = p_rc.tile([128, 8], F16, tag="rc2")
            with nc.allow_low_precision(reason="fp16 softmax denom"):
                nc.vector.reciprocal(rc2[:], sm2[:])
            r2dram = p_dram.tile([1, 1024], F16, tag="r2dram")
            nc.sync.dma_start(
                r2dram[:].rearrange("a (p c) -> (a p) c", p=128), rc2[:],
                single_packet=True,
            )
            rbc = p_rc.tile([128, 1024], F16, tag="rbc")
            nc.sync.dma_start(rbc[:], r2dram[0:1, :].to_broadcast((128, 1024)))
            yield None
            # halves, so a waiting multiply never blocks the DVE queue long
            obh = p_out.tile([128, 1024], F16, tag="oh", name=f"obh{h}{qb}")
            for hf in range(2):
                sl = slice(hf * 512, (hf + 1) * 512)
                nc.vector.tensor_mul(obh[:, sl], ob[:, sl], rbc[:, sl])
                nc.sync.dma_start(
                    out_ap[h * 128:(h + 1) * 128,
                           qb * 1024 + hf * 512:qb * 1024 + (hf + 1) * 512],
                    obh[:, sl],
                )
                yield None

        def norm_final(blk):
            """Tail chain for the very last block: row sums straight into
            [128, 8] via tiny stationary matmuls over the (k<=13) fold plus
            et14/et15 (PE is idle here and the fold chain lags ~2us), one
            unshuffle DMA, then a PE broadcast — minimizes serial DMAs."""
            h, qb, acc, pv = blk.h, blk.qb, blk.acc, blk.pv
            smq = ps_pj.tile([128, 512], F32, tag="proj", name="smq")
            srcs = [acc, blk.acc_o, blk.ets[14], blk.ets[15]]
            for j in range(8):
                # stationary column m is q = j*128 + m: smq[p, j] holds
                # rowsum(q = j*128 + p)
                for si, src in enumerate(srcs):
                    nc.tensor.matmul(
                        smq[:, j:j + 1], src[:, j * 128:(j + 1) * 128], ones[:],
                        start=(si == 0), stop=(si == len(srcs) - 1),
                        skip_group_check=True,
                    )
            rc2 = p_rc.tile([128, 8], F16, tag="rc2")
            with nc.allow_low_precision(reason="fp16 softmax denom"):
                nc.vector.reciprocal(rc2[:], smq[:, 0:8])
            # stay on-chip: PE transpose + per-block broadcast matmuls skip
            # the ~2.5us of DMA fixed costs an unshuffle round trip takes
            tps = ps_pj.tile([128, 512], F32, tag="proj", name="tps")
            t16 = tps[:].bitcast(F16)
            nc.tensor.transpose(t16[0:8, 0:128], rc2[:], ident[:])
            t_sb = p_rc.tile([8, 128], F16, tag="t_sb")
            nc.vector.tensor_copy(t_sb[:], t16[0:8, 0:128])
            # stage pv in SBUF (DVE may read only one PSUM input); halves,
            # emitted after the reciprocal so they don't delay it on DVE
            ob_pv = p_out.tile([128, 1024], F32, tag="o", name="ob_pv")
            nc.vector.tensor_copy(ob_pv[:, 0:512], pv[:, 0:512])
            rbc_ps = ps_mm.tile([128, 1024], F32, tag="sT", name="rbc_ps")
            for j in range(8):
                # sel[:, j-block] is the row-j indicator: out = T[j, :] bcast
                nc.tensor.matmul(
                    rbc_ps[:, j * 128:(j + 1) * 128],
                    sel8[:, j * 128:(j + 1) * 128], t_sb[:],
                    start=True, stop=True, skip_group_check=True,
                )
            for hf in range(2):
                sl = slice(hf * 512, (hf + 1) * 512)
                if hf == 1:
                    nc.vector.tensor_copy(ob_pv[:, 512:1024], pv[:, 512:1024])
                ob = p_out.tile([128, 512], F16, tag="of", name=f"of{hf}")
                nc.vector.tensor_mul(ob[:], ob_pv[:, sl], rbc_ps[:, sl])
                nc.sync.dma_start(
                    out_ap[h * 128:(h + 1) * 128,
                           qb * 1024 + hf * 512:qb * 1024 + (hf + 1) * 512],
                    ob[:],
                )

        # head 0's projections run serially (nothing to hide them under) and
        # share the phase-A PSUM pool, so they don't wait on its release;
        # heads 1..3 project inside earlier blocks' attention loops.
        q0_, k0_, gen = proj_steps(0, ps_v, copy_alt=True, tag="v")
        for _ in range(33):     # 8 chunks x 4 yields + 1: the final copy
            next(gen)           # is only emitted on the next() AFTER the
                                # last chunk's 4th yield
        ps_v.release()
        ps_mm = ctx.enter_context(tc.tile_pool(name="psmm", bufs=2, space="PSUM"))
        ps_pv = ctx.enter_context(tc.tile_pool(name="pspv", bufs=1, space="PSUM"))

        with tc.tile_pool(name="pspj", bufs=2, space="PSUM") as ps_pj:
            qts, kts = {0: q0_}, {0: k0_}
            projs = {}

            def make_proj(h):
                qth, kth, g = proj_steps(h, ps_pj)
                qts[h], kts[h], projs[h] = qth, kth, g

            make_proj(1)
            blocks = [Blk(b // 2, b % 2) for b in range(8)]
            blocks[7].final = True
            # per-block drip-feed plan:
            #   proj[b] = (head whose projection is injected, total pairs)
            #   pre[b]  = how many qk+exp steps of block b+1 to pre-execute
            # Each exp is 1038ns on ACT vs 852ns of matching qk+pv on PE, so
            # blocks with no injected work go ACT-bound. Shifting every
            # projection one block early lets each block pre-execute the
            # next one's first qk+exp steps (the cascade below), and head
            # 3's V projection fills block 5.
            # 33 next()s per generator: the final copy is emitted on the
            # call after the last chunk's 4th yield
            proj_plan = {0: (1, 33), 1: (2, 16), 2: (2, 17), 3: (3, 16),
                         4: (3, 17)}
            pre_plan = {3: 2, 4: 7, 5: 12, 6: 6}
            norm_in = None

            def emit_v3_chunk(sp):
                # block 5 has no projection in flight, so the proj ring is free
                ps = ps_pj.tile([128, 512], F32, tag="proj", name=f"v3_{sp}")
                for j in range(2):
                    st = sp * 2 + j
                    for d in range(ND):
                        nc.tensor.matmul(
                            ps[:, j * 128:(j + 1) * 128],
                            xts(d, st // 4)[:, (st % 4) * 128:(st % 4 + 1) * 128],
                            wsl("wv", d)[:, V012:DHG],
                            start=(d == 0),
                            stop=(d == ND - 1),
                            skip_group_check=True,
                        )
                vt = p_v.tile([128, 256], F16, tag="v3", bufs=8,
                              name=f"vt3_{sp}")
                nc.vector.tensor_copy(vt[:], ps[:, 0:256])
                vts3[sp] = vt

            for b, blk in enumerate(blocks):
                h = blk.h
                qt, kt = qts[h], kts[h]
                blk.pv = ps_pv.tile([128, 1024], F32, tag="pv",
                                    name=f"pv{h}_{blk.qb}")
                k0 = len(blk.ets)
                kq, kp, it = k0, 0, 0
                pre_left = pre_plan.get(b, 0)
                ph, pairs_left = proj_plan.get(b, (None, 0))
                if ph is not None and ph not in projs:
                    make_proj(ph)
                nproj = projs.get(ph)
                v3_left = 8 if b == 5 else 0

                if kq < KT:
                    qk_step(blk, qt, kt, kq)
                    kq += 1
                if norm_in is not None:
                    next(norm_in, None)         # pv-freeing copy
                if kq < KT:
                    qk_step(blk, qt, kt, kq)
                    kq += 1
                if norm_in is not None:
                    next(norm_in, None)         # row-sum matmuls
                if k0 > 0 and norm_in is not None:
                    next(norm_in, None)         # reciprocal DMA chain

                while kp < KT:
                    if k0 > 0:
                        # pre-filled block: pv leads so the exp-ring WAR
                        # order stays correct (pv(k) must be emitted before
                        # qk(k + ring) reuses et(k)'s slot)
                        pv_step(blk, kp)
                        kp += 1
                        if kq < KT:
                            qk_step(blk, qt, kt, kq)
                            kq += 1
                    elif kq < KT:
                        qk_step(blk, qt, kt, kq)
                        kq += 1
                        if kq - 2 >= kp:
                            pv_step(blk, kp)
                            kp += 1
                    else:
                        pv_step(blk, kp)
                        kp += 1
                    it += 1
                    if it == 1 and k0 == 0 and norm_in is not None:
                        next(norm_in, None)     # reciprocal DMA chain
                    if it == 5 and norm_in is not None:
                        # broadcast is in flight by now; the multiplies
                        # won't head-of-line-block the DVE queue for long
                        for _ in norm_in:
                            pass
                        norm_in = None
                    iters_left = max(1, 15 - it)
                    n_inj = min(pairs_left, 2,
                                (pairs_left + iters_left - 1) // iters_left)
                    for _ in range(n_inj):
                        next(nproj)
                        pairs_left -= 1
                    if pre_left > 0 and it >= 2:
                        nblk = blocks[b + 1]
                        pk = len(nblk.ets)
                        qk_step(nblk, qts[nblk.h], kts[nblk.h], pk, pre=True)
                        pre_left -= 1
                    if v3_left > 0 and it % 2 == 0:
                        emit_v3_chunk(8 - v3_left)
                        v3_left -= 1
                while pairs_left > 0:
                    next(nproj)
                    pairs_left -= 1
                while v3_left > 0:
                    emit_v3_chunk(8 - v3_left)
                    v3_left -= 1
                if b < 7:
                    norm_in = norm_steps(blk, ps_pj)
                else:
                    if norm_in is not None:
                        for _ in norm_in:
                            pass
                        norm_in = None
                    norm_final(blk)


def _build():
    nc = bacc.Bacc(
        "TRN2",
        target_bir_lowering=False,
        debug=False,
        enable_asserts=False,
        num_devices=N_CORES,
    )
    xt_ap = nc.dram_tensor("xt", [D, S], F16, kind="ExternalInput").ap()
    wq_ap = nc.dram_tensor("wq", [128, ND * DHG], F16, kind="ExternalInput").ap()
    wk_ap = nc.dram_tensor("wk", [128, ND * DHG], F16, kind="ExternalInput").ap()
    wv_ap = nc.dram_tensor("wv", [128, ND * DHG], F16, kind="ExternalInput").ap()
    sel_ap = nc.dram_tensor("sel", [8, 1024], F16, kind="ExternalInput").ap()
    out_ap = nc.dram_tensor("out", [DHG, S], F16, kind="ExternalOutput").ap()
    with tile.TileContext(nc) as tc:
        _emit(tc, nc, xt_ap, wq_ap, wk_ap, wv_ap, sel_ap, out_ap)
    nc.compile()
    return nc


def _shard_inputs(inputs):
    x = np.ascontiguousarray(np.asarray(inputs["input_embeddings"], dtype=np.float32))
    wq = np.asarray(inputs["w_query"], dtype=np.float32) * SCALE
    wk = np.asarray(inputs["w_key"], dtype=np.float32)
    wv = np.asarray(inputs["w_value"], dtype=np.float32)

    def gather(w, g):
        # head h occupies the strided cols d = hd*8 + h; regroup head-major,
        # then d-major so each [128, 1024] DMA chunk is a plain slice
        w4 = w.reshape(D, DH, H)[:, :, g * HPC:(g + 1) * HPC]   # (D, hd, hl)
        wg = w4.transpose(0, 2, 1).reshape(ND, 128, DHG)
        return np.ascontiguousarray(
            wg.transpose(1, 0, 2).reshape(128, ND * DHG).astype(np.float16)
        )

    sel = np.kron(np.eye(8), np.ones((1, 128))).astype(np.float16)
    in_maps = []
    for c in range(N_CORES):
        b, g = divmod(c, 2)
        in_maps.append(
            {
                "xt": np.ascontiguousarray(x[b].T.astype(np.float16)),
                "wq": gather(wq, g),
                "wk": gather(wk, g),
                "wv": gather(wv, g),
                "sel": sel,
            }
        )
    return in_maps


def kernel(**inputs):
    nc = _CACHE.get("nc")
    if nc is None:
        nc = _CACHE["nc"] = _build()
    in_maps = _shard_inputs(inputs)
    res = run_bass_kernel_spmd(
        nc, in_maps, core_ids=list(range(N_CORES)), trace=TRACE
    )
    _CACHE["last_result"] = res
    out = np.empty((B, S, DH, H), dtype=np.float32)
    for c in range(N_CORES):
        b, g = divmod(c, 2)
        o = res.results[c]["out"].reshape(HPC, DH, S)            # (hl, hd, s)
        out[b, :, :, g * HPC:(g + 1) * HPC] = o.transpose(2, 1, 0)
    return out.reshape(B, S, D)


# revision 87
# speedup vs baseline: 1.0070x; 1.0031x over previous
"""Multi-head self-attention (B=4, S=2048, D=1024, H=8) on 8 TRN2 NeuronCores.

Sharding: core c -> batch b=c//2, head-group g=c%2 (4 heads/core).
Each core computes its 4 heads' attention output [512, 2048] (transposed,
head-major); the host gathers/reassembles the full [B, S, D] output.

Notes on the math: the reference adds the source mask per-QUERY (constant
along the key axis) before a softmax over keys, so the mask cancels exactly;
encoder_output_embedding and the target mask are unused by the reference.
The kernel therefore computes pure softmax(q k^T / sqrt(dh)) v, with the
1/sqrt(dh) scale folded into w_query on the host.

Schedule (per core):
  A) V = x @ wv, d-outer over 8 PSUM banks so the first matmul only waits
     on ~2 DMA chunks and the d-loop streams behind the DMA queue (the
     HWDGE processes one descriptor set per ~625ns, so inputs arrive as
     28 x 256KB chunks, not 56 x 128KB).
  B) head 0 q/k projection (PE-serial; nothing to hide it under).
  C) per head: flash-style attention with the next head's projection
     matmuls drip-fed into the ACT-paced inner loop. ACT (exp) has slack
     in heads 0-2 but is the binding engine in head 3, so head 2 hosts
     head 3's full projection in its first block and pre-executes the
     first 8 QK+exp steps of head 3's first block in its second; head 3's
     first block pre-executes 3 exp steps of the second. Row sums fold on
     DVE right after each exp; the softmax denominator pipeline overlaps
     the trailing PV matmuls and the next block's QK.
"""

import math
from contextlib import ExitStack

import numpy as np

import concourse.bacc as bacc
import concourse.tile as tile
from concourse import masks, mybir
from concourse.bass_utils import run_bass_kernel_spmd

N_CORES = 8
B, S, D, H = 4, 2048, 1024, 8
DH = 128                    # head dim
HPC = 4                     # heads per core
DHG = HPC * DH              # 512: projected width per core
SCALE = 1.0 / math.sqrt(DH)
KT = S // 128               # 16 key tiles
ND = D // 128               # 8 contraction tiles
NSB = S // 512              # 4 column blocks of x

F32 = mybir.dt.float32
F16 = mybir.dt.float16

TRACE = False               # test.py flips this for profiling runs
_CACHE = {}


def _emit(tc, nc, xt_ap, wq_ap, wk_ap, wv_ap, sel_ap, out_ap):
    with ExitStack() as ctx:
        p_xt = ctx.enter_context(tc.tile_pool(name="xt", bufs=16))
        p_w = ctx.enter_context(tc.tile_pool(name="w", bufs=4))
        # 3 live per tag: head h-1 still being read by its last block while
        # head h is read and head h+1 is being projected (plan shifts the
        # projections one block early)
        p_qt = ctx.enter_context(tc.tile_pool(name="qt", bufs=3))
        p_v = ctx.enter_context(tc.tile_pool(name="v", bufs=KT))
        p_exp = ctx.enter_context(tc.tile_pool(name="exp", bufs=6))
        p_out = ctx.enter_context(tc.tile_pool(name="o", bufs=2))
        p_rc = ctx.enter_context(tc.tile_pool(name="rc", bufs=2))
        p_const = ctx.enter_context(tc.tile_pool(name="const", bufs=1))
        p_dram = ctx.enter_context(tc.tile_pool(name="dram", bufs=2, space="DRAM"))

        ones = p_const.tile([128, 1], F16, tag="ones")
        nc.vector.memset(ones[:], 1.0)
        ones_row = p_const.tile([1, 128], F16, tag="ones_row")
        nc.vector.memset(ones_row[:], 1.0)
        ident = p_const.tile([128, 128], F16, tag="ident")
        masks.make_identity(nc, ident[:])
        sel8 = p_const.tile([8, 1024], F16, tag="sel8")

        # DMA chunking: [128, 1024] chunks (2KB/partition) halve the count
        # of 625ns HWDGE descriptor slots vs per-tile loads. xh[d][half]
        # covers x^T rows d*128.. cols half*1024..; wc[name][dp] packs two
        # 128-row weight chunks side by side.
        xh = [[None, None] for _ in range(ND)]
        wc = {"wv": [None] * 4, "wq": [None] * 4, "wk": [None] * 4}

        fine = {}                       # (kind, idx): [128, 512] head tiles

        def xts(d, sb):
            t = fine.get(("x", d, sb))
            if t is not None:
                return t[:]
            return xh[d][sb // 2][:, (sb % 2) * 512:(sb % 2) * 512 + 512]

        def wsl(name, d):
            t = fine.get((name, d))
            if t is not None:
                return t[:]
            return wc[name][d // 2][:, (d % 2) * DHG:(d % 2) * DHG + DHG]

        def dma_x(d, half):
            t = p_xt.tile([128, 1024], F16, tag="xt", name=f"x{d}_{half}")
            nc.sync.dma_start(
                t[:], xt_ap[d * 128:(d + 1) * 128, half * 1024:(half + 1) * 1024]
            )
            xh[d][half] = t

        def dma_w(name, ap, dp):
            # host lays weights out d-major: ap is [128, ND*DHG]
            t = p_w.tile([128, 2 * DHG], F16, tag=name, name=f"{name}{dp}")
            nc.sync.dma_start(t[:], ap[:, dp * 2 * DHG:(dp + 1) * 2 * DHG])
            wc[name][dp] = t

        # the first matmul needs only (wv[0], x[0, sb0]): issue those as
        # [128, 512] singles so PE starts ~1us sooner
        for d0 in range(2):
            t = p_w.tile([128, DHG], F16, tag="wvf", bufs=2, name=f"wvf{d0}")
            nc.sync.dma_start(t[:], wv_ap[:, d0 * DHG:(d0 + 1) * DHG])
            fine[("wv", d0)] = t
            t = p_xt.tile([128, 512], F16, tag="xtf", bufs=2, name=f"xf{d0}")
            nc.sync.dma_start(t[:], xt_ap[d0 * 128:(d0 + 1) * 128, 0:512])
            fine[("x", d0, 0)] = t
        for dp in range(4):
            if dp > 0:
                dma_w("wv", wv_ap, dp)
            dma_x(2 * dp, 0)
            dma_x(2 * dp + 1, 0)
        for dp in range(4):
            dma_w("wq", wq_ap, dp)
            dma_x(2 * dp, 1)
            dma_x(2 * dp + 1, 1)
        for dp in range(4):
            dma_w("wk", wk_ap, dp)
        nc.sync.dma_start(sel8[:], sel_ap[:, :])   # only needed at the tail

        # ---- Phase A: V(heads 0-2) = x @ wv[:, :384], d-outer over PSUM --
        # head 3's V columns are deferred into block 5 (head2-qb1), which
        # otherwise has no projection work to hide its ACT-bound exp loop.
        V012 = 3 * DH
        vts = [None] * KT
        vts3 = [None] * (KT // 2)       # head-3 V, [128, 256] per st-pair
        ps_v = tc.alloc_tile_pool(name="psv", bufs=8, space="PSUM")
        if True:
            for wave in range(2):
                pss = [
                    ps_v.tile([128, 512], F32, tag="v", name=f"psv{wave}_{i}")
                    for i in range(8)
                ]
                for d in range(ND):
                    for i in range(8):
                        st = wave * 8 + i
                        nc.tensor.matmul(
                            pss[i][:, 0:V012],
                            xts(d, st // 4)[:, (st % 4) * 128:(st % 4 + 1) * 128],
                            wsl("wv", d)[:, 0:V012],
                            start=(d == 0),
                            stop=(d == ND - 1),
                        )
                for i in range(8):
                    st = wave * 8 + i
                    vt = p_v.tile([128, V012], F16, tag="v", name=f"vt{st}")
                    # alternate engines so the copies drain in half the time
                    # and wave 2's first matmuls aren't blocked on bank reuse
                    if i % 2 == 0:
                        nc.scalar.activation(
                            vt[:], pss[i][:, 0:V012],
                            mybir.ActivationFunctionType.Copy,
                        )
                    else:
                        nc.vector.tensor_copy(vt[:], pss[i][:, 0:V012])
                    vts[st] = vt

        # ---- Attention-phase PSUM pools (4 + 2 + 2 = 8 banks) ----------
        def proj_steps(h, pool, copy_alt=False, tag="proj"):
            """Create head h's q/k tiles; return (qt, kt, generator) where
            the generator emits one 2-matmul chunk per next()."""
            qt = p_qt.tile([128, S], F16, tag="qt", name=f"qt{h}")
            kt = p_qt.tile([128, S], F16, tag="kt", name=f"kt{h}")

            def gen():
                n = 0
                order = [(qt, "wq", 0), (qt, "wq", 1), (kt, "wk", 0),
                         (kt, "wk", 1), (kt, "wk", 2), (kt, "wk", 3),
                         (qt, "wq", 2), (qt, "wq", 3)]
                for dst, wname, sb in order:
                    if True:
                        ps = pool.tile([128, 512], F32, tag=tag,
                                       name=f"pj{h}_{wname}{sb}")
                        for d in range(ND):
                            nc.tensor.matmul(
                                ps[:],
                                wsl(wname, d)[:, h * 128:(h + 1) * 128],
                                xts(d, sb),
                                start=(d == 0),
                                stop=(d == ND - 1),
                            )
                            if d % 2 == 1:
                                yield None
                        dsl = dst[:, sb * 512:(sb + 1) * 512]
                        if copy_alt and n % 2 == 0:
                            nc.scalar.activation(
                                dsl, ps[:], mybir.ActivationFunctionType.Copy
                            )
                        else:
                            nc.vector.tensor_copy(dsl, ps[:])
                        n += 1
                while True:
                    yield None

            return qt, kt, gen()

        # per-block attention state shared between qk/pv/norm emitters
        class Blk:
            def __init__(self, h, qb):
                self.h, self.qb = h, qb
                self.q0 = qb * 1024
                self.ets = {}
                self.acc = None
                self.pv = None
                self.final = False

        def qk_step(blk, qt, kt, k, pre=False):
            st_ps = ps_mm.tile([128, 1024], F32, tag="sT",
                               name=f"sT{blk.h}_{blk.qb}_{k}")
            for hf in range(2):
                nc.tensor.matmul(
                    st_ps[:, hf * 512:(hf + 1) * 512],
                    kt[:, k * 128:(k + 1) * 128],
                    qt[:, blk.q0 + hf * 512:blk.q0 + (hf + 1) * 512],
                    start=True,
                    stop=True,
                )
            et = p_exp.tile([128, 1024], F16, tag="pre" if pre else "exp",
                            bufs=14 if pre else 9, name=f"et{blk.h}_{blk.qb}_{k}")
            nc.scalar.activation(et[:], st_ps[:], mybir.ActivationFunctionType.Exp)
            blk.ets[k] = et
            # fold the row-sum accumulator as soon as exp lands, so the
            # denominator chain starts before the last PV
            if blk.final:
                # two parallel fold chains (even/odd k through 13) so no
                # serial DVE chain lags the tail; et14/et15 are summed by
                # the row-sum matmuls directly
                if k == 2:
                    acc = p_exp.tile([128, 1024], F16, tag="acc", bufs=5,
                                     name="acc_e")
                    nc.vector.tensor_add(acc[:], blk.ets[0][:], et[:])
                    blk.acc = acc
                elif k == 3:
                    acc = p_exp.tile([128, 1024], F16, tag="acc", bufs=5,
                                     name="acc_o")
                    nc.vector.tensor_add(acc[:], blk.ets[1][:], et[:])
                    blk.acc_o = acc
                elif 4 <= k <= 13:
                    dst = blk.acc if k % 2 == 0 else blk.acc_o
                    nc.vector.tensor_add(dst[:], dst[:], et[:])
            elif k == 1:
                acc = p_exp.tile([128, 1024], F16, tag="acc", bufs=5,
                                 name=f"acc{blk.h}_{blk.qb}")
                nc.vector.tensor_add(acc[:], blk.ets[0][:], et[:])
                blk.acc = acc
            elif k > 1:
                nc.vector.tensor_add(blk.acc[:], blk.acc[:], et[:])

        def pv_step(blk, k):
            if blk.final and k >= 14:
                et = blk.ets[k]         # norm_final still needs it
            else:
                et = blk.ets.pop(k)
            if blk.h < 3:
                vsl = vts[k][:, blk.h * 128:(blk.h + 1) * 128]
            else:
                vsl = vts3[k // 2][:, (k % 2) * 128:(k % 2 + 1) * 128]
            for hf in range(2):
                sl = slice(hf * 512, (hf + 1) * 512)
                nc.tensor.matmul(
                    blk.pv[:, sl],
                    vsl,
                    et[:, sl],
                    start=(k == 0),
                    stop=(k == KT - 1),
                )

        def norm_steps(blk, ps_pj):
            """Softmax-denominator chain for a non-final block; yields so the
            caller interleaves it with the next block's emission."""
            h, qb, acc, pv = blk.h, blk.qb, blk.acc, blk.pv
            # free the pv PSUM bank right away — the next block's first PV
            # matmul sits behind this chain in PE queue order
            ob = p_out.tile([128, 1024], F32, tag="o", name=f"ob{h}{qb}")
            nc.vector.tensor_copy(ob[:], pv[:])
            yield None
            sms = []
            for hf in range(2):
                sm = ps_pj.tile([1, 512], F32, tag="proj", name=f"sm{h}{qb}{hf}")
                nc.tensor.matmul(
                    sm[:], ones[:], acc[:, hf * 512:(hf + 1) * 512],
                    start=True, stop=True,
                )
                sms.append(sm)
            yield None
            sm_sb = p_rc.tile([1, 1024], F32, tag="sm_sb")
            for hf in range(2):
                nc.vector.tensor_copy(sm_sb[:, hf * 512:(hf + 1) * 512], sms[hf][:])
            sm2 = p_rc.tile([128, 8], F32, tag="sm2")
            nc.sync.dma_start(sm2[:], sm_sb[:], single_packet=True)
            rc2 = p_rc.tile([128, 8], F16, tag="rc2")
            with nc.allow_low_precision(reason="fp16 softmax denom"):
                nc.vector.reciprocal(rc2[:], sm2[:])
            r2dram = p_dram.tile([1, 1024], F16, tag="r2dram")
            nc.sync.dma_start(
                r2dram[:].rearrange("a (p c) -> (a p) c", p=128), rc2[:],
                single_packet=True,
            )
            rbc = p_rc.tile([128, 1024], F16, tag="rbc")
            nc.sync.dma_start(rbc[:], r2dram[0:1, :].to_broadcast((128, 1024)))
            yield None
            # halves, so a waiting multiply never blocks the DVE queue long
            obh = p_out.tile([128, 1024], F16, tag="oh", name=f"obh{h}{qb}")
            for hf in range(2):
                sl = slice(hf * 512, (hf + 1) * 512)
                nc.vector.tensor_mul(obh[:, sl], ob[:, sl], rbc[:, sl])
                nc.sync.dma_start(
                    out_ap[h * 128:(h + 1) * 128,
                           qb * 1024 + hf * 512:qb * 1024 + (hf + 1) * 512],
                    obh[:, sl],
                )
                yield None

        def norm_fast(blk, ps_pj):
            """DMA-less denominator chain for the second-to-last block: the
            3-DMA chain takes ~8us and would land mid-final-block, colliding
            with the tail; direct row-sum matmuls + PE transpose + selector
            broadcasts retire it early instead."""
            h, qb, acc, pv = blk.h, blk.qb, blk.acc, blk.pv
            ob = p_out.tile([128, 1024], F32, tag="o", name=f"obf{h}{qb}")
            nc.vector.tensor_copy(ob[:], pv[:])
            yield None
            smq = ps_pj.tile([128, 512], F32, tag="proj", name=f"smqf{h}{qb}")
            for j in range(8):
                nc.tensor.matmul(
                    smq[:, j:j + 1], acc[:, j * 128:(j + 1) * 128], ones[:],
                    start=True, stop=True, skip_group_check=True,
                )
            yield None
            rc2 = p_rc.tile([128, 8], F16, tag="rc2")
            with nc.allow_low_precision(reason="fp16 softmax denom"):
                nc.vector.reciprocal(rc2[:], smq[:, 0:8])
            yield None
            tps = ps_pj.tile([128, 512], F32, tag="proj", name=f"tpsf{h}{qb}")
            t16 = tps[:].bitcast(F16)
            nc.tensor.transpose(t16[0:8, 0:128], rc2[:], ident[:])
            t_sb = p_rc.tile([8, 128], F16, tag="t_sb")
            nc.vector.tensor_copy(t_sb[:], t16[0:8, 0:128])
            for hf in range(2):
                rbc = ps_pj.tile([128, 512], F32, tag="proj",
                                 name=f"rbcf{h}{qb}{hf}")
                for j in range(4):
                    jj = hf * 4 + j
                    nc.tensor.matmul(
                        rbc[:, j * 128:(j + 1) * 128],
                        sel8[:, jj * 128:(jj + 1) * 128], t_sb[:],
                        start=True, stop=True, skip_group_check=True,
                    )
                obh = p_out.tile([128, 512], F16, tag="of",
                                 name=f"obf2{h}{qb}{hf}")
                nc.vector.tensor_mul(
                    obh[:], ob[:, hf * 512:(hf + 1) * 512], rbc[:]
                )
                nc.sync.dma_start(
                    out_ap[h * 128:(h + 1) * 128,
                           qb * 1024 + hf * 512:qb * 1024 + (hf + 1) * 512],
                    obh[:],
                )
                yield None

        def norm_final(blk):
            """Tail chain for the very last block: row sums straight into
            [128, 8] via tiny stationary matmuls over the (k<=13) fold plus
            et14/et15 (PE is idle here and the fold chain lags ~2us), one
            unshuffle DMA, then a PE broadcast — minimizes serial DMAs."""
            h, qb, acc, pv = blk.h, blk.qb, blk.acc, blk.pv
            smq = ps_pj.tile([128, 512], F32, tag="proj", name="smq")
            srcs = [acc, blk.acc_o, blk.ets[14], blk.ets[15]]
            for j in range(8):
                # stationary column m is q = j*128 + m: smq[p, j] holds
                # rowsum(q = j*128 + p)
                for si, src in enumerate(srcs):
                    nc.tensor.matmul(
                        smq[:, j:j + 1], src[:, j * 128:(j + 1) * 128], ones[:],
                        start=(si == 0), stop=(si == len(srcs) - 1),
                        skip_group_check=True,
                    )
            rc2 = p_rc.tile([128, 8], F16, tag="rc2")
            with nc.allow_low_precision(reason="fp16 softmax denom"):
                nc.vector.reciprocal(rc2[:], smq[:, 0:8])
            # stay on-chip: PE transpose + per-block broadcast matmuls skip
            # the ~2.5us of DMA fixed costs an unshuffle round trip takes
            tps = ps_pj.tile([128, 512], F32, tag="proj", name="tps")
            t16 = tps[:].bitcast(F16)
            nc.tensor.transpose(t16[0:8, 0:128], rc2[:], ident[:])
            t_sb = p_rc.tile([8, 128], F16, tag="t_sb")
            nc.vector.tensor_copy(t_sb[:], t16[0:8, 0:128])
            # stage pv in SBUF (DVE may read only one PSUM input); halves,
            # emitted after the reciprocal so they don't delay it on DVE
            ob_pv = p_out.tile([128, 1024], F32, tag="o", name="ob_pv")
            nc.vector.tensor_copy(ob_pv[:, 0:512], pv[:, 0:512])
            rbc_ps = ps_mm.tile([128, 1024], F32, tag="sT", name="rbc_ps")
            for j in range(8):
                # sel[:, j-block] is the row-j indicator: out = T[j, :] bcast
                nc.tensor.matmul(
                    rbc_ps[:, j * 128:(j + 1) * 128],
                    sel8[:, j * 128:(j + 1) * 128], t_sb[:],
                    start=True, stop=True, skip_group_check=True,
                )
            for hf in range(2):
                sl = slice(hf * 512, (hf + 1) * 512)
                if hf == 1:
                    nc.vector.tensor_copy(ob_pv[:, 512:1024], pv[:, 512:1024])
                ob = p_out.tile([128, 512], F16, tag="of", name=f"of{hf}")
                nc.vector.tensor_mul(ob[:], ob_pv[:, sl], rbc_ps[:, sl])
                nc.sync.dma_start(
                    out_ap[h * 128:(h + 1) * 128,
                           qb * 1024 + hf * 512:qb * 1024 + (hf + 1) * 512],
                    ob[:],
                )

        # head 0's projections run serially (nothing to hide them under) and
        # share the phase-A PSUM pool, so they don't wait on its release;
        # heads 1..3 project inside earlier blocks' attention loops.
        q0_, k0_, gen = proj_steps(0, ps_v, copy_alt=True, tag="v")
        for _ in range(33):     # 8 chunks x 4 yields + 1: the final copy
            next(gen)           # is only emitted on the next() AFTER the
                                # last chunk's 4th yield
        ps_v.release()
        ps_mm = ctx.enter_context(tc.tile_pool(name="psmm", bufs=2, space="PSUM"))
        ps_pv = ctx.enter_context(tc.tile_pool(name="pspv", bufs=1, space="PSUM"))

        with tc.tile_pool(name="pspj", bufs=2, space="PSUM") as ps_pj:
            qts, kts = {0: q0_}, {0: k0_}
            projs = {}

            def make_proj(h):
                qth, kth, g = proj_steps(h, ps_pj)
                qts[h], kts[h], projs[h] = qth, kth, g

            make_proj(1)
            blocks = [Blk(b // 2, b % 2) for b in range(8)]
            blocks[7].final = True
            # per-block drip-feed plan:
            #   proj[b] = (head whose projection is injected, total pairs)
            #   pre[b]  = how many qk+exp steps of block b+1 to pre-execute
            # Each exp is 1038ns on ACT vs 852ns of matching qk+pv on PE, so
            # blocks with no injected work go ACT-bound. Shifting every
            # projection one block early lets each block pre-execute the
            # next one's first qk+exp steps (the cascade below), and head
            # 3's V projection fills block 5.
            # 33 next()s per generator: the final copy is emitted on the
            # call after the last chunk's 4th yield
            proj_plan = {0: (1, 33), 1: (2, 16), 2: (2, 17), 3: (3, 16),
                         4: (3, 17)}
            pre_plan = {3: 2, 4: 7, 5: 12, 6: 6}
            norm_in = None

            def emit_v3_chunk(sp):
                # block 5 has no projection in flight, so the proj ring is free
                ps = ps_pj.tile([128, 512], F32, tag="proj", name=f"v3_{sp}")
                for j in range(2):
                    st = sp * 2 + j
                    for d in range(ND):
                        nc.tensor.matmul(
                            ps[:, j * 128:(j + 1) * 128],
                            xts(d, st // 4)[:, (st % 4) * 128:(st % 4 + 1) * 128],
                            wsl("wv", d)[:, V012:DHG],
                            start=(d == 0),
                            stop=(d == ND - 1),
                            skip_group_check=True,
                        )
                vt = p_v.tile([128, 256], F16, tag="v3", bufs=8,
                              name=f"vt3_{sp}")
                nc.vector.tensor_copy(vt[:], ps[:, 0:256])
                vts3[sp] = vt

            for b, blk in enumerate(blocks):
                h = blk.h
                qt, kt = qts[h], kts[h]
                blk.pv = ps_pv.tile([128, 1024], F32, tag="pv",
                                    name=f"pv{h}_{blk.qb}")
                k0 = len(blk.ets)
                kq, kp, it = k0, 0, 0
                pre_left = pre_plan.get(b, 0)
                ph, pairs_left = proj_plan.get(b, (None, 0))
                if ph is not None and ph not in projs:
                    make_proj(ph)
                nproj = projs.get(ph)
                v3_left = 8 if b == 5 else 0

                if kq < KT:
                    qk_step(blk, qt, kt, kq)
                    kq += 1
                if norm_in is not None:
                    next(norm_in, None)         # pv-freeing copy
                if kq < KT:
                    qk_step(blk, qt, kt, kq)
                    kq += 1
                if norm_in is not None:
                    next(norm_in, None)         # row-sum matmuls
                if k0 > 0 and norm_in is not None:
                    next(norm_in, None)         # reciprocal DMA chain

                while kp < KT:
                    if k0 > 0:
                        # pre-filled block: pv leads so the exp-ring WAR
                        # order stays correct (pv(k) must be emitted before
                        # qk(k + ring) reuses et(k)'s slot)
                        pv_step(blk, kp)
                        kp += 1
                        if kq < KT:
                            qk_step(blk, qt, kt, kq)
                            kq += 1
                    elif kq < KT:
                        qk_step(blk, qt, kt, kq)
                        kq += 1
                        if kq - 2 >= kp:
                            pv_step(blk, kp)
                            kp += 1
                    else:
                        pv_step(blk, kp)
                        kp += 1
                    it += 1
                    if it == 1 and k0 == 0 and norm_in is not None:
                        next(norm_in, None)     # reciprocal DMA chain
                    if it == 5 and norm_in is not None:
                        # broadcast is in flight by now; the multiplies
                        # won't head-of-line-block the DVE queue for long
                        for _ in norm_in:
                            pass
                        norm_in = None
                    iters_left = max(1, 15 - it)
                    n_inj = min(pairs_left, 2,
                                (pairs_left + iters_left - 1) // iters_left)
                    for _ in range(n_inj):
                        next(nproj)
                        pairs_left -= 1
                    if pre_left > 0 and it >= 2:
                        nblk = blocks[b + 1]
                        pk = len(nblk.ets)
                        qk_step(nblk, qts[nblk.h], kts[nblk.h], pk, pre=True)
                        pre_left -= 1
                    if v3_left > 0 and it % 2 == 0:
                        emit_v3_chunk(8 - v3_left)
                        v3_left -= 1
                while pairs_left > 0:
                    next(nproj)
                    pairs_left -= 1
                while v3_left > 0:
                    emit_v3_chunk(8 - v3_left)
                    v3_left -= 1
                if b == 6:
                    norm_in = norm_fast(blk, ps_pj)
                elif b < 7:
                    norm_in = norm_steps(blk, ps_pj)
                else:
                    if norm_in is not None:
                        for _ in norm_in:
                            pass
                        norm_in = None
                    norm_final(blk)


def _build():
    nc = bacc.Bacc(
        "TRN2",
        target_bir_lowering=False,
        debug=False,
        enable_asserts=False,
        num_devices=N_CORES,
    )
    xt_ap = nc.dram_tensor("xt", [D, S], F16, kind="ExternalInput").ap()
    wq_ap = nc.dram_tensor("wq", [128, ND * DHG], F16, kind="ExternalInput").ap()
    wk_ap = nc.dram_tensor("wk", [128, ND * DHG], F16, kind="ExternalInput").ap()
    wv_ap = nc.dram_tensor("wv", [128, ND * DHG], F16, kind="ExternalInput").ap()
    sel_ap = nc.dram_tensor("sel", [8, 1024], F16, kind="ExternalInput").ap()
    out_ap = nc.dram_tensor("out", [DHG, S], F16, kind="ExternalOutput").ap()
    with tile.TileContext(nc) as tc:
        _emit(tc, nc, xt_ap, wq_ap, wk_ap, wv_ap, sel_ap, out_ap)
    nc.compile()
    return nc


def _shard_inputs(inputs):
    x = np.ascontiguousarray(np.asarray(inputs["input_embeddings"], dtype=np.float32))
    wq = np.asarray(inputs["w_query"], dtype=np.float32) * SCALE
    wk = np.asarray(inputs["w_key"], dtype=np.float32)
    wv = np.asarray(inputs["w_value"], dtype=np.float32)

    def gather(w, g):
        # head h occupies the strided cols d = hd*8 + h; regroup head-major,
        # then d-major so each [128, 1024] DMA chunk is a plain slice
        w4 = w.reshape(D, DH, H)[:, :, g * HPC:(g + 1) * HPC]   # (D, hd, hl)
        wg = w4.transpose(0, 2, 1).reshape(ND, 128, DHG)
        return np.ascontiguousarray(
            wg.transpose(1, 0, 2).reshape(128, ND * DHG).astype(np.float16)
        )

    sel = np.kron(np.eye(8), np.ones((1, 128))).astype(np.float16)
    in_maps = []
    for c in range(N_CORES):
        b, g = divmod(c, 2)
        in_maps.append(
            {
                "xt": np.ascontiguousarray(x[b].T.astype(np.float16)),
                "wq": gather(wq, g),
                "wk": gather(wk, g),
                "wv": gather(wv, g),
                "sel": sel,
            }
        )
    return in_maps


def kernel(**inputs):
    nc = _CACHE.get("nc")
    if nc is None:
        nc = _CACHE["nc"] = _build()
    in_maps = _shard_inputs(inputs)
    res = run_bass_kernel_spmd(
        nc, in_maps, core_ids=list(range(N_CORES)), trace=TRACE
    )
    _CACHE["last_result"] = res
    out = np.empty((B, S, DH, H), dtype=np.float32)
    for c in range(N_CORES):
        b, g = divmod(c, 2)
        o = res.results[c]["out"].reshape(HPC, DH, S)            # (hl, hd, s)
        out[b, :, :, g * HPC:(g + 1) * HPC] = o.transpose(2, 1, 0)
    return out.reshape(B, S, D)


# revision 88
# speedup vs baseline: 1.0093x; 1.0023x over previous
"""Multi-head self-attention (B=4, S=2048, D=1024, H=8) on 8 TRN2 NeuronCores.

Sharding: core c -> batch b=c//2, head-group g=c%2 (4 heads/core).
Each core computes its 4 heads' attention output [512, 2048] (transposed,
head-major); the host gathers/reassembles the full [B, S, D] output.

Notes on the math: the reference adds the source mask per-QUERY (constant
along the key axis) before a softmax over keys, so the mask cancels exactly;
encoder_output_embedding and the target mask are unused by the reference.
The kernel therefore computes pure softmax(q k^T / sqrt(dh)) v, with the
1/sqrt(dh) scale folded into w_query on the host.

Schedule (per core):
  A) V = x @ wv, d-outer over 8 PSUM banks so the first matmul only waits
     on ~2 DMA chunks and the d-loop streams behind the DMA queue (the
     HWDGE processes one descriptor set per ~625ns, so inputs arrive as
     28 x 256KB chunks, not 56 x 128KB).
  B) head 0 q/k projection (PE-serial; nothing to hide it under).
  C) per head: flash-style attention with the next head's projection
     matmuls drip-fed into the ACT-paced inner loop. ACT (exp) has slack
     in heads 0-2 but is the binding engine in head 3, so head 2 hosts
     head 3's full projection in its first block and pre-executes the
     first 8 QK+exp steps of head 3's first block in its second; head 3's
     first block pre-executes 3 exp steps of the second. Row sums fold on
     DVE right after each exp; the softmax denominator pipeline overlaps
     the trailing PV matmuls and the next block's QK.
"""

import math
from contextlib import ExitStack

import numpy as np

import concourse.bacc as bacc
import concourse.tile as tile
from concourse import masks, mybir
from concourse.bass_utils import run_bass_kernel_spmd

N_CORES = 8
B, S, D, H = 4, 2048, 1024, 8
DH = 128                    # head dim
HPC = 4                     # heads per core
DHG = HPC * DH              # 512: projected width per core
SCALE = 1.0 / math.sqrt(DH)
KT = S // 128               # 16 key tiles
ND = D // 128               # 8 contraction tiles
NSB = S // 512              # 4 column blocks of x

F32 = mybir.dt.float32
F16 = mybir.dt.float16

TRACE = False               # test.py flips this for profiling runs
_CACHE = {}


def _emit(tc, nc, xt_ap, wq_ap, wk_ap, wv_ap, sel_ap, out_ap):
    with ExitStack() as ctx:
        p_xt = ctx.enter_context(tc.tile_pool(name="xt", bufs=16))
        p_w = ctx.enter_context(tc.tile_pool(name="w", bufs=4))
        # 3 live per tag: head h-1 still being read by its last block while
        # head h is read and head h+1 is being projected (plan shifts the
        # projections one block early)
        p_qt = ctx.enter_context(tc.tile_pool(name="qt", bufs=3))
        p_v = ctx.enter_context(tc.tile_pool(name="v", bufs=KT))
        p_exp = ctx.enter_context(tc.tile_pool(name="exp", bufs=6))
        p_out = ctx.enter_context(tc.tile_pool(name="o", bufs=2))
        p_rc = ctx.enter_context(tc.tile_pool(name="rc", bufs=2))
        p_const = ctx.enter_context(tc.tile_pool(name="const", bufs=1))
        p_dram = ctx.enter_context(tc.tile_pool(name="dram", bufs=2, space="DRAM"))

        ones = p_const.tile([128, 1], F16, tag="ones")
        nc.vector.memset(ones[:], 1.0)
        ones_row = p_const.tile([1, 128], F16, tag="ones_row")
        nc.vector.memset(ones_row[:], 1.0)
        ident = p_const.tile([128, 128], F16, tag="ident")
        masks.make_identity(nc, ident[:])
        sel8 = p_const.tile([8, 1024], F16, tag="sel8")

        # DMA chunking: [128, 1024] chunks (2KB/partition) halve the count
        # of 625ns HWDGE descriptor slots vs per-tile loads. xh[d][half]
        # covers x^T rows d*128.. cols half*1024..; wc[name][dp] packs two
        # 128-row weight chunks side by side.
        xh = [[None, None] for _ in range(ND)]
        wc = {"wv": [None] * 4, "wq": [None] * 4, "wk": [None] * 4}

        fine = {}                       # (kind, idx): [128, 512] head tiles

        def xts(d, sb):
            t = fine.get(("x", d, sb))
            if t is not None:
                return t[:]
            return xh[d][sb // 2][:, (sb % 2) * 512:(sb % 2) * 512 + 512]

        def wsl(name, d):
            t = fine.get((name, d))
            if t is not None:
                return t[:]
            return wc[name][d // 2][:, (d % 2) * DHG:(d % 2) * DHG + DHG]

        def dma_x(d, half):
            t = p_xt.tile([128, 1024], F16, tag="xt", name=f"x{d}_{half}")
            nc.sync.dma_start(
                t[:], xt_ap[d * 128:(d + 1) * 128, half * 1024:(half + 1) * 1024]
            )
            xh[d][half] = t

        def dma_w(name, ap, dp):
            # host lays weights out d-major: ap is [128, ND*DHG]
            t = p_w.tile([128, 2 * DHG], F16, tag=name, name=f"{name}{dp}")
            nc.sync.dma_start(t[:], ap[:, dp * 2 * DHG:(dp + 1) * 2 * DHG])
            wc[name][dp] = t

        # the first matmul needs only (wv[0], x[0, sb0]): issue those as
        # [128, 512] singles so PE starts ~1us sooner
        for d0 in range(2):
            t = p_w.tile([128, DHG], F16, tag="wvf", bufs=2, name=f"wvf{d0}")
            nc.sync.dma_start(t[:], wv_ap[:, d0 * DHG:(d0 + 1) * DHG])
            fine[("wv", d0)] = t
            t = p_xt.tile([128, 512], F16, tag="xtf", bufs=2, name=f"xf{d0}")
            nc.sync.dma_start(t[:], xt_ap[d0 * 128:(d0 + 1) * 128, 0:512])
            fine[("x", d0, 0)] = t
        for dp in range(4):
            if dp > 0:
                dma_w("wv", wv_ap, dp)
            dma_x(2 * dp, 0)
            dma_x(2 * dp + 1, 0)
        for dp in range(4):
            dma_w("wq", wq_ap, dp)
            dma_x(2 * dp, 1)
            dma_x(2 * dp + 1, 1)
        for dp in range(4):
            dma_w("wk", wk_ap, dp)
        nc.sync.dma_start(sel8[:], sel_ap[:, :])   # only needed at the tail

        # ---- Phase A: V(heads 0-2) = x @ wv[:, :384], d-outer over PSUM --
        # head 3's V columns are deferred into block 5 (head2-qb1), which
        # otherwise has no projection work to hide its ACT-bound exp loop.
        V012 = 3 * DH
        vts = [None] * KT
        vts3 = [None] * (KT // 2)       # head-3 V, [128, 256] per st-pair
        ps_v = tc.alloc_tile_pool(name="psv", bufs=8, space="PSUM")
        if True:
            for wave in range(2):
                pss = [
                    ps_v.tile([128, 512], F32, tag="v", name=f"psv{wave}_{i}")
                    for i in range(8)
                ]
                for d in range(ND):
                    for i in range(8):
                        st = wave * 8 + i
                        nc.tensor.matmul(
                            pss[i][:, 0:V012],
                            xts(d, st // 4)[:, (st % 4) * 128:(st % 4 + 1) * 128],
                            wsl("wv", d)[:, 0:V012],
                            start=(d == 0),
                            stop=(d == ND - 1),
                        )
                for i in range(8):
                    st = wave * 8 + i
                    vt = p_v.tile([128, V012], F16, tag="v", name=f"vt{st}")
                    # alternate engines so the copies drain in half the time
                    # and wave 2's first matmuls aren't blocked on bank reuse
                    if i % 2 == 0:
                        nc.scalar.activation(
                            vt[:], pss[i][:, 0:V012],
                            mybir.ActivationFunctionType.Copy,
                        )
                    else:
                        nc.vector.tensor_copy(vt[:], pss[i][:, 0:V012])
                    vts[st] = vt

        # ---- Attention-phase PSUM pools (4 + 2 + 2 = 8 banks) ----------
        def proj_steps(h, pool, copy_alt=False, tag="proj"):
            """Create head h's q/k tiles; return (qt, kt, generator) where
            the generator emits one 2-matmul chunk per next()."""
            qt = p_qt.tile([128, S], F16, tag="qt", name=f"qt{h}")
            kt = p_qt.tile([128, S], F16, tag="kt", name=f"kt{h}")

            def gen():
                n = 0
                order = [(qt, "wq", 0), (qt, "wq", 1), (kt, "wk", 0),
                         (kt, "wk", 1), (kt, "wk", 2), (kt, "wk", 3),
                         (qt, "wq", 2), (qt, "wq", 3)]
                for dst, wname, sb in order:
                    if True:
                        ps = pool.tile([128, 512], F32, tag=tag,
                                       name=f"pj{h}_{wname}{sb}")
                        for d in range(ND):
                            nc.tensor.matmul(
                                ps[:],
                                wsl(wname, d)[:, h * 128:(h + 1) * 128],
                                xts(d, sb),
                                start=(d == 0),
                                stop=(d == ND - 1),
                            )
                            if d % 2 == 1:
                                yield None
                        dsl = dst[:, sb * 512:(sb + 1) * 512]
                        if copy_alt and n % 2 == 0:
                            nc.scalar.activation(
                                dsl, ps[:], mybir.ActivationFunctionType.Copy
                            )
                        else:
                            nc.vector.tensor_copy(dsl, ps[:])
                        n += 1
                while True:
                    yield None

            return qt, kt, gen()

        # per-block attention state shared between qk/pv/norm emitters
        class Blk:
            def __init__(self, h, qb):
                self.h, self.qb = h, qb
                self.q0 = qb * 1024
                self.ets = {}
                self.acc = None
                self.pv = None
                self.final = False

        def qk_step(blk, qt, kt, k, pre=False):
            st_ps = ps_mm.tile([128, 1024], F32, tag="sT",
                               name=f"sT{blk.h}_{blk.qb}_{k}")
            for hf in range(2):
                nc.tensor.matmul(
                    st_ps[:, hf * 512:(hf + 1) * 512],
                    kt[:, k * 128:(k + 1) * 128],
                    qt[:, blk.q0 + hf * 512:blk.q0 + (hf + 1) * 512],
                    start=True,
                    stop=True,
                )
            et = p_exp.tile([128, 1024], F16, tag="pre" if pre else "exp",
                            bufs=14 if pre else 9, name=f"et{blk.h}_{blk.qb}_{k}")
            nc.scalar.activation(et[:], st_ps[:], mybir.ActivationFunctionType.Exp)
            blk.ets[k] = et
            # fold the row-sum accumulator as soon as exp lands, so the
            # denominator chain starts before the last PV
            if blk.final:
                # two parallel fold chains (even/odd k through 13) so no
                # serial DVE chain lags the tail; et14/et15 are summed by
                # the row-sum matmuls directly
                if k == 2:
                    acc = p_exp.tile([128, 1024], F16, tag="acc", bufs=5,
                                     name="acc_e")
                    nc.vector.tensor_add(acc[:], blk.ets[0][:], et[:])
                    blk.acc = acc
                elif k == 3:
                    acc = p_exp.tile([128, 1024], F16, tag="acc", bufs=5,
                                     name="acc_o")
                    nc.vector.tensor_add(acc[:], blk.ets[1][:], et[:])
                    blk.acc_o = acc
                elif 4 <= k <= 13:
                    dst = blk.acc if k % 2 == 0 else blk.acc_o
                    nc.vector.tensor_add(dst[:], dst[:], et[:])
            elif k == 1:
                acc = p_exp.tile([128, 1024], F16, tag="acc", bufs=5,
                                 name=f"acc{blk.h}_{blk.qb}")
                nc.vector.tensor_add(acc[:], blk.ets[0][:], et[:])
                blk.acc = acc
            elif k > 1:
                nc.vector.tensor_add(blk.acc[:], blk.acc[:], et[:])

        def pv_step(blk, k):
            if blk.final and k >= 14:
                et = blk.ets[k]         # norm_final still needs it
            else:
                et = blk.ets.pop(k)
            if blk.h < 3:
                vsl = vts[k][:, blk.h * 128:(blk.h + 1) * 128]
            else:
                vsl = vts3[k // 2][:, (k % 2) * 128:(k % 2 + 1) * 128]
            for hf in range(2):
                sl = slice(hf * 512, (hf + 1) * 512)
                nc.tensor.matmul(
                    blk.pv[:, sl],
                    vsl,
                    et[:, sl],
                    start=(k == 0),
                    stop=(k == KT - 1),
                )

        def norm_steps(blk, ps_pj):
            """Softmax-denominator chain for a non-final block; yields so the
            caller interleaves it with the next block's emission."""
            h, qb, acc, pv = blk.h, blk.qb, blk.acc, blk.pv
            # free the pv PSUM bank right away — the next block's first PV
            # matmul sits behind this chain in PE queue order
            ob = p_out.tile([128, 1024], F32, tag="o", name=f"ob{h}{qb}")
            nc.vector.tensor_copy(ob[:], pv[:])
            yield None
            sms = []
            for hf in range(2):
                sm = ps_pj.tile([1, 512], F32, tag="proj", name=f"sm{h}{qb}{hf}")
                nc.tensor.matmul(
                    sm[:], ones[:], acc[:, hf * 512:(hf + 1) * 512],
                    start=True, stop=True,
                )
                sms.append(sm)
            yield None
            sm_sb = p_rc.tile([1, 1024], F32, tag="sm_sb")
            for hf in range(2):
                nc.vector.tensor_copy(sm_sb[:, hf * 512:(hf + 1) * 512], sms[hf][:])
            sm2 = p_rc.tile([128, 8], F32, tag="sm2")
            nc.sync.dma_start(sm2[:], sm_sb[:], single_packet=True)
            rc2 = p_rc.tile([128, 8], F16, tag="rc2")
            with nc.allow_low_precision(reason="fp16 softmax denom"):
                nc.vector.reciprocal(rc2[:], sm2[:])
            r2dram = p_dram.tile([1, 1024], F16, tag="r2dram")
            nc.sync.dma_start(
                r2dram[:].rearrange("a (p c) -> (a p) c", p=128), rc2[:],
                single_packet=True,
            )
            rbc = p_rc.tile([128, 1024], F16, tag="rbc")
            nc.sync.dma_start(rbc[:], r2dram[0:1, :].to_broadcast((128, 1024)))
            yield None
            # halves, so a waiting multiply never blocks the DVE queue long
            obh = p_out.tile([128, 1024], F16, tag="oh", name=f"obh{h}{qb}")
            for hf in range(2):
                sl = slice(hf * 512, (hf + 1) * 512)
                nc.vector.tensor_mul(obh[:, sl], ob[:, sl], rbc[:, sl])
                nc.sync.dma_start(
                    out_ap[h * 128:(h + 1) * 128,
                           qb * 1024 + hf * 512:qb * 1024 + (hf + 1) * 512],
                    obh[:, sl],
                )
                yield None

        def norm_fast(blk, ps_pj):
            """DMA-less denominator chain for the second-to-last block: the
            3-DMA chain takes ~8us and would land mid-final-block, colliding
            with the tail; direct row-sum matmuls + PE transpose + selector
            broadcasts retire it early instead."""
            h, qb, acc, pv = blk.h, blk.qb, blk.acc, blk.pv
            ob = p_out.tile([128, 1024], F32, tag="o", name=f"obf{h}{qb}")
            nc.vector.tensor_copy(ob[:], pv[:])
            yield None
            smq = ps_pj.tile([128, 512], F32, tag="proj", name=f"smqf{h}{qb}")
            for j in range(8):
                nc.tensor.matmul(
                    smq[:, j:j + 1], acc[:, j * 128:(j + 1) * 128], ones[:],
                    start=True, stop=True, skip_group_check=True,
                )
            yield None
            rc2 = p_rc.tile([128, 8], F16, tag="rc2")
            with nc.allow_low_precision(reason="fp16 softmax denom"):
                nc.vector.reciprocal(rc2[:], smq[:, 0:8])
            yield None
            tps = ps_pj.tile([128, 512], F32, tag="proj", name=f"tpsf{h}{qb}")
            t16 = tps[:].bitcast(F16)
            nc.tensor.transpose(t16[0:8, 0:128], rc2[:], ident[:])
            t_sb = p_rc.tile([8, 128], F16, tag="t_sb")
            nc.vector.tensor_copy(t_sb[:], t16[0:8, 0:128])
            for hf in range(2):
                rbc = ps_pj.tile([128, 512], F32, tag="proj",
                                 name=f"rbcf{h}{qb}{hf}")
                for j in range(4):
                    jj = hf * 4 + j
                    nc.tensor.matmul(
                        rbc[:, j * 128:(j + 1) * 128],
                        sel8[:, jj * 128:(jj + 1) * 128], t_sb[:],
                        start=True, stop=True, skip_group_check=True,
                    )
                obh = p_out.tile([128, 512], F16, tag="of",
                                 name=f"obf2{h}{qb}{hf}")
                nc.vector.tensor_mul(
                    obh[:], ob[:, hf * 512:(hf + 1) * 512], rbc[:]
                )
                nc.sync.dma_start(
                    out_ap[h * 128:(h + 1) * 128,
                           qb * 1024 + hf * 512:qb * 1024 + (hf + 1) * 512],
                    obh[:],
                )
                yield None

        def norm_final(blk):
            """Tail chain for the very last block: row sums straight into
            [128, 8] via tiny stationary matmuls over the (k<=13) fold plus
            et14/et15 (PE is idle here and the fold chain lags ~2us), one
            unshuffle DMA, then a PE broadcast — minimizes serial DMAs."""
            h, qb, acc, pv = blk.h, blk.qb, blk.acc, blk.pv
            smq = ps_pj.tile([128, 512], F32, tag="proj", name="smq")
            srcs = [acc, blk.acc_o, blk.ets[14], blk.ets[15]]
            for j in range(8):
                # stationary column m is q = j*128 + m: smq[p, j] holds
                # rowsum(q = j*128 + p)
                for si, src in enumerate(srcs):
                    nc.tensor.matmul(
                        smq[:, j:j + 1], src[:, j * 128:(j + 1) * 128], ones[:],
                        start=(si == 0), stop=(si == len(srcs) - 1),
                        skip_group_check=True,
                    )
            rc2 = p_rc.tile([128, 8], F16, tag="rc2")
            with nc.allow_low_precision(reason="fp16 softmax denom"):
                nc.vector.reciprocal(rc2[:], smq[:, 0:8])
            # stay on-chip: PE transpose + per-block broadcast matmuls skip
            # the ~2.5us of DMA fixed costs an unshuffle round trip takes
            tps = ps_pj.tile([128, 512], F32, tag="proj", name="tps")
            t16 = tps[:].bitcast(F16)
            nc.tensor.transpose(t16[0:8, 0:128], rc2[:], ident[:])
            t_sb = p_rc.tile([8, 128], F16, tag="t_sb")
            nc.vector.tensor_copy(t_sb[:], t16[0:8, 0:128])
            # stage pv in SBUF (DVE may read only one PSUM input); halves,
            # emitted after the reciprocal so they don't delay it on DVE
            ob_pv = p_out.tile([128, 1024], F32, tag="o", name="ob_pv")
            nc.vector.tensor_copy(ob_pv[:, 0:512], pv[:, 0:512])
            rbc_ps = ps_mm.tile([128, 1024], F32, tag="sT", name="rbc_ps")
            for j in range(8):
                # sel[:, j-block] is the row-j indicator: out = T[j, :] bcast
                nc.tensor.matmul(
                    rbc_ps[:, j * 128:(j + 1) * 128],
                    sel8[:, j * 128:(j + 1) * 128], t_sb[:],
                    start=True, stop=True, skip_group_check=True,
                )
            for hf in range(2):
                sl = slice(hf * 512, (hf + 1) * 512)
                if hf == 1:
                    nc.vector.tensor_copy(ob_pv[:, 512:1024], pv[:, 512:1024])
                ob = p_out.tile([128, 512], F16, tag="of", name=f"of{hf}")
                nc.vector.tensor_mul(ob[:], ob_pv[:, sl], rbc_ps[:, sl])
                nc.sync.dma_start(
                    out_ap[h * 128:(h + 1) * 128,
                           qb * 1024 + hf * 512:qb * 1024 + (hf + 1) * 512],
                    ob[:],
                )

        # head 0's projections run serially (nothing to hide them under) and
        # share the phase-A PSUM pool, so they don't wait on its release;
        # heads 1..3 project inside earlier blocks' attention loops.
        q0_, k0_, gen = proj_steps(0, ps_v, copy_alt=True, tag="v")
        for _ in range(33):     # 8 chunks x 4 yields + 1: the final copy
            next(gen)           # is only emitted on the next() AFTER the
                                # last chunk's 4th yield
        ps_v.release()
        ps_mm = ctx.enter_context(tc.tile_pool(name="psmm", bufs=2, space="PSUM"))
        ps_pv = ctx.enter_context(tc.tile_pool(name="pspv", bufs=1, space="PSUM"))

        with tc.tile_pool(name="pspj", bufs=2, space="PSUM") as ps_pj:
            qts, kts = {0: q0_}, {0: k0_}
            projs = {}

            def make_proj(h):
                qth, kth, g = proj_steps(h, ps_pj)
                qts[h], kts[h], projs[h] = qth, kth, g

            make_proj(1)
            blocks = [Blk(b // 2, b % 2) for b in range(8)]
            blocks[7].final = True
            # per-block drip-feed plan:
            #   proj[b] = (head whose projection is injected, total pairs)
            #   pre[b]  = how many qk+exp steps of block b+1 to pre-execute
            # Each exp is 1038ns on ACT vs 852ns of matching qk+pv on PE, so
            # blocks with no injected work go ACT-bound. Shifting every
            # projection one block early lets each block pre-execute the
            # next one's first qk+exp steps (the cascade below), and head
            # 3's V projection fills block 5.
            # 33 next()s per generator: the final copy is emitted on the
            # call after the last chunk's 4th yield
            proj_plan = {0: (1, 33), 1: (2, 16), 2: (2, 17), 3: (3, 16),
                         4: (3, 17)}
            pre_plan = {3: 2, 4: 7, 5: 12, 6: 6}
            norm_in = None

            def emit_v3_chunk(sp):
                # block 5 has no projection in flight, so the proj ring is free
                ps = ps_pj.tile([128, 512], F32, tag="proj", name=f"v3_{sp}")
                for j in range(2):
                    st = sp * 2 + j
                    for d in range(ND):
                        nc.tensor.matmul(
                            ps[:, j * 128:(j + 1) * 128],
                            xts(d, st // 4)[:, (st % 4) * 128:(st % 4 + 1) * 128],
                            wsl("wv", d)[:, V012:DHG],
                            start=(d == 0),
                            stop=(d == ND - 1),
                            skip_group_check=True,
                        )
                vt = p_v.tile([128, 256], F16, tag="v3", bufs=8,
                              name=f"vt3_{sp}")
                nc.vector.tensor_copy(vt[:], ps[:, 0:256])
                vts3[sp] = vt

            for b, blk in enumerate(blocks):
                h = blk.h
                qt, kt = qts[h], kts[h]
                blk.pv = ps_pv.tile([128, 1024], F32, tag="pv",
                                    name=f"pv{h}_{blk.qb}")
                k0 = len(blk.ets)
                kq, kp, it = k0, 0, 0
                pre_left = pre_plan.get(b, 0)
                ph, pairs_left = proj_plan.get(b, (None, 0))
                if ph is not None and ph not in projs:
                    make_proj(ph)
                nproj = projs.get(ph)
                v3_left = 8 if b == 5 else 0

                if kq < KT:
                    qk_step(blk, qt, kt, kq)
                    kq += 1
                if norm_in is not None:
                    next(norm_in, None)         # pv-freeing copy
                if kq < KT:
                    qk_step(blk, qt, kt, kq)
                    kq += 1
                if norm_in is not None:
                    next(norm_in, None)         # row-sum matmuls
                if k0 > 0 and norm_in is not None:
                    next(norm_in, None)         # reciprocal DMA chain

                while kp < KT:
                    if k0 > 0:
                        # pre-filled block: pv leads so the exp-ring WAR
                        # order stays correct (pv(k) must be emitted before
                        # qk(k + ring) reuses et(k)'s slot)
                        pv_step(blk, kp)
                        kp += 1
                        if kq < KT:
                            qk_step(blk, qt, kt, kq)
                            kq += 1
                    elif kq < KT:
                        qk_step(blk, qt, kt, kq)
                        kq += 1
                        if kq - 2 >= kp:
                            pv_step(blk, kp)
                            kp += 1
                    else:
                        pv_step(blk, kp)
                        kp += 1
                    it += 1
                    if it == 1 and k0 == 0 and norm_in is not None:
                        next(norm_in, None)     # reciprocal DMA chain
                    if it == 5 and norm_in is not None:
                        # broadcast is in flight by now; the multiplies
                        # won't head-of-line-block the DVE queue for long
                        for _ in norm_in:
                            pass
                        norm_in = None
                    iters_left = max(1, 15 - it)
                    n_inj = min(pairs_left, 2,
                                (pairs_left + iters_left - 1) // iters_left)
                    for _ in range(n_inj):
                        next(nproj)
                        pairs_left -= 1
                    if pre_left > 0 and it >= 2:
                        nblk = blocks[b + 1]
                        pk = len(nblk.ets)
                        qk_step(nblk, qts[nblk.h], kts[nblk.h], pk, pre=True)
                        pre_left -= 1
                    if v3_left > 0 and it % 2 == 0:
                        emit_v3_chunk(8 - v3_left)
                        v3_left -= 1
                while pairs_left > 0:
                    next(nproj)
                    pairs_left -= 1
                while v3_left > 0:
                    emit_v3_chunk(8 - v3_left)
                    v3_left -= 1
                if b in (5, 6):
                    norm_in = norm_fast(blk, ps_pj)
                elif b < 7:
                    norm_in = norm_steps(blk, ps_pj)
                else:
                    if norm_in is not None:
                        for _ in norm_in:
                            pass
                        norm_in = None
                    norm_final(blk)


def _build():
    nc = bacc.Bacc(
        "TRN2",
        target_bir_lowering=False,
        debug=False,
        enable_asserts=False,
        num_devices=N_CORES,
    )
    xt_ap = nc.dram_tensor("xt", [D, S], F16, kind="ExternalInput").ap()
    wq_ap = nc.dram_tensor("wq", [128, ND * DHG], F16, kind="ExternalInput").ap()
    wk_ap = nc.dram_tensor("wk", [128, ND * DHG], F16, kind="ExternalInput").ap()
    wv_ap = nc.dram_tensor("wv", [128, ND * DHG], F16, kind="ExternalInput").ap()
    sel_ap = nc.dram_tensor("sel", [8, 1024], F16, kind="ExternalInput").ap()
    out_ap = nc.dram_tensor("out", [DHG, S], F16, kind="ExternalOutput").ap()
    with tile.TileContext(nc) as tc:
        _emit(tc, nc, xt_ap, wq_ap, wk_ap, wv_ap, sel_ap, out_ap)
    nc.compile()
    return nc


def _shard_inputs(inputs):
    x = np.ascontiguousarray(np.asarray(inputs["input_embeddings"], dtype=np.float32))
    wq = np.asarray(inputs["w_query"], dtype=np.float32) * SCALE
    wk = np.asarray(inputs["w_key"], dtype=np.float32)
    wv = np.asarray(inputs["w_value"], dtype=np.float32)

    def gather(w, g):
        # head h occupies the strided cols d = hd*8 + h; regroup head-major,
        # then d-major so each [128, 1024] DMA chunk is a plain slice
        w4 = w.reshape(D, DH, H)[:, :, g * HPC:(g + 1) * HPC]   # (D, hd, hl)
        wg = w4.transpose(0, 2, 1).reshape(ND, 128, DHG)
        return np.ascontiguousarray(
            wg.transpose(1, 0, 2).reshape(128, ND * DHG).astype(np.float16)
        )

    sel = np.kron(np.eye(8), np.ones((1, 128))).astype(np.float16)
    in_maps = []
    for c in range(N_CORES):
        b, g = divmod(c, 2)
        in_maps.append(
            {
                "xt": np.ascontiguousarray(x[b].T.astype(np.float16)),
                "wq": gather(wq, g),
                "wk": gather(wk, g),
                "wv": gather(wv, g),
                "sel": sel,
            }
        )
    return in_maps


def kernel(**inputs):
    nc = _CACHE.get("nc")
    if nc is None:
        nc = _CACHE["nc"] = _build()
    in_maps = _shard_inputs(inputs)
    res = run_bass_kernel_spmd(
        nc, in_maps, core_ids=list(range(N_CORES)), trace=TRACE
    )
    _CACHE["last_result"] = res
    out = np.empty((B, S, DH, H), dtype=np.float32)
    for c in range(N_CORES):
        b, g = divmod(c, 2)
        o = res.results[c]["out"].reshape(HPC, DH, S)            # (hl, hd, s)
        out[b, :, :, g * HPC:(g + 1) * HPC] = o.transpose(2, 1, 0)
    return out.reshape(B, S, D)


# revision 100
# speedup vs baseline: 1.0152x; 1.0058x over previous
"""Multi-head self-attention (B=4, S=2048, D=1024, H=8) on 8 TRN2 NeuronCores.

Sharding: core c -> batch b=c//2, head-group g=c%2 (4 heads/core).
Each core computes its 4 heads' attention output [512, 2048] (transposed,
head-major); the host gathers/reassembles the full [B, S, D] output.

Notes on the math: the reference adds the source mask per-QUERY (constant
along the key axis) before a softmax over keys, so the mask cancels exactly;
encoder_output_embedding and the target mask are unused by the reference.
The kernel therefore computes pure softmax(q k^T / sqrt(dh)) v, with the
1/sqrt(dh) scale folded into w_query on the host.

Schedule (per core):
  A) V = x @ wv, d-outer over 8 PSUM banks so the first matmul only waits
     on ~2 DMA chunks and the d-loop streams behind the DMA queue (the
     HWDGE processes one descriptor set per ~625ns, so inputs arrive as
     28 x 256KB chunks, not 56 x 128KB).
  B) head 0 q/k projection (PE-serial; nothing to hide it under).
  C) per head: flash-style attention with the next head's projection
     matmuls drip-fed into the ACT-paced inner loop. ACT (exp) has slack
     in heads 0-2 but is the binding engine in head 3, so head 2 hosts
     head 3's full projection in its first block and pre-executes the
     first 8 QK+exp steps of head 3's first block in its second; head 3's
     first block pre-executes 3 exp steps of the second. Row sums fold on
     DVE right after each exp; the softmax denominator pipeline overlaps
     the trailing PV matmuls and the next block's QK.
"""

import math
from contextlib import ExitStack

import numpy as np

import concourse.bacc as bacc
import concourse.tile as tile
from concourse import masks, mybir
from concourse.bass_utils import run_bass_kernel_spmd

N_CORES = 8
B, S, D, H = 4, 2048, 1024, 8
DH = 128                    # head dim
HPC = 4                     # heads per core
DHG = HPC * DH              # 512: projected width per core
SCALE = 1.0 / math.sqrt(DH)
KT = S // 128               # 16 key tiles
ND = D // 128               # 8 contraction tiles
NSB = S // 512              # 4 column blocks of x

F32 = mybir.dt.float32
F16 = mybir.dt.float16

TRACE = False               # test.py flips this for profiling runs
_CACHE = {}


def _emit(tc, nc, xt_ap, wq_ap, wk_ap, wv_ap, sel_ap, out_ap):
    with ExitStack() as ctx:
        p_xt = ctx.enter_context(tc.tile_pool(name="xt", bufs=16))
        p_w = ctx.enter_context(tc.tile_pool(name="w", bufs=4))
        # 3 live per tag: head h-1 still being read by its last block while
        # head h is read and head h+1 is being projected (plan shifts the
        # projections one block early)
        p_qt = ctx.enter_context(tc.tile_pool(name="qt", bufs=3))
        p_v = ctx.enter_context(tc.tile_pool(name="v", bufs=KT))
        p_exp = ctx.enter_context(tc.tile_pool(name="exp", bufs=6))
        p_out = ctx.enter_context(tc.tile_pool(name="o", bufs=2))
        p_rc = ctx.enter_context(tc.tile_pool(name="rc", bufs=2))
        p_const = ctx.enter_context(tc.tile_pool(name="const", bufs=1))
        p_dram = ctx.enter_context(tc.tile_pool(name="dram", bufs=2, space="DRAM"))

        ones = p_const.tile([128, 1], F16, tag="ones")
        nc.vector.memset(ones[:], 1.0)
        ones_row = p_const.tile([1, 128], F16, tag="ones_row")
        nc.vector.memset(ones_row[:], 1.0)
        ident = p_const.tile([128, 128], F16, tag="ident")
        masks.make_identity(nc, ident[:])
        sel8 = p_const.tile([8, 1024], F16, tag="sel8")

        # DMA chunking: [128, 1024] chunks (2KB/partition) halve the count
        # of 625ns HWDGE descriptor slots vs per-tile loads. xh[d][half]
        # covers x^T rows d*128.. cols half*1024..; wc[name][dp] packs two
        # 128-row weight chunks side by side.
        xh = [[None, None] for _ in range(ND)]
        wc = {"wv": [None] * 4, "wq": [None] * 4, "wk": [None] * 4}

        fine = {}                       # (kind, idx): [128, 512] head tiles

        def xts(d, sb):
            t = fine.get(("x", d, sb))
            if t is not None:
                return t[:]
            return xh[d][sb // 2][:, (sb % 2) * 512:(sb % 2) * 512 + 512]

        def wsl(name, d):
            t = fine.get((name, d))
            if t is not None:
                return t[:]
            return wc[name][d // 2][:, (d % 2) * DHG:(d % 2) * DHG + DHG]

        def dma_x(d, half):
            t = p_xt.tile([128, 1024], F16, tag="xt", name=f"x{d}_{half}")
            nc.sync.dma_start(
                t[:], xt_ap[d * 128:(d + 1) * 128, half * 1024:(half + 1) * 1024]
            )
            xh[d][half] = t

        def dma_w(name, ap, dp):
            # host lays weights out d-major: ap is [128, ND*DHG]
            t = p_w.tile([128, 2 * DHG], F16, tag=name, name=f"{name}{dp}")
            nc.sync.dma_start(t[:], ap[:, dp * 2 * DHG:(dp + 1) * 2 * DHG])
            wc[name][dp] = t

        # the first matmul needs only (wv[0], x[0, sb0]): issue those as
        # [128, 512] singles so PE starts ~1us sooner
        for d0 in range(2):
            t = p_w.tile([128, DHG], F16, tag="wvf", bufs=2, name=f"wvf{d0}")
            nc.sync.dma_start(t[:], wv_ap[:, d0 * DHG:(d0 + 1) * DHG])
            fine[("wv", d0)] = t
            t = p_xt.tile([128, 512], F16, tag="xtf", bufs=2, name=f"xf{d0}")
            nc.sync.dma_start(t[:], xt_ap[d0 * 128:(d0 + 1) * 128, 0:512])
            fine[("x", d0, 0)] = t
        for dp in range(4):
            if dp > 0:
                dma_w("wv", wv_ap, dp)
            dma_x(2 * dp, 0)
            dma_x(2 * dp + 1, 0)
        for dp in range(4):
            dma_w("wq", wq_ap, dp)
            dma_x(2 * dp, 1)
            dma_x(2 * dp + 1, 1)
        for dp in range(4):
            dma_w("wk", wk_ap, dp)
        nc.sync.dma_start(sel8[:], sel_ap[:, :])   # only needed at the tail

        # ---- Phase A: V(heads 0-2) = x @ wv[:, :384], d-outer over PSUM --
        # head 3's V columns are deferred into block 5 (head2-qb1), which
        # otherwise has no projection work to hide its ACT-bound exp loop.
        V012 = 3 * DH
        vts = [None] * KT
        vts3 = [None] * (KT // 2)       # head-3 V, [128, 256] per st-pair
        ps_v = tc.alloc_tile_pool(name="psv", bufs=8, space="PSUM")
        if True:
            for wave in range(2):
                pss = [
                    ps_v.tile([128, 512], F32, tag="v", name=f"psv{wave}_{i}")
                    for i in range(8)
                ]
                for d in range(ND):
                    for i in range(8):
                        st = wave * 8 + i
                        nc.tensor.matmul(
                            pss[i][:, 0:V012],
                            xts(d, st // 4)[:, (st % 4) * 128:(st % 4 + 1) * 128],
                            wsl("wv", d)[:, 0:V012],
                            start=(d == 0),
                            stop=(d == ND - 1),
                        )
                for i in range(8):
                    st = wave * 8 + i
                    vt = p_v.tile([128, V012], F16, tag="v", name=f"vt{st}")
                    # alternate engines so the copies drain in half the time
                    # and wave 2's first matmuls aren't blocked on bank reuse
                    if i % 2 == 0:
                        nc.scalar.activation(
                            vt[:], pss[i][:, 0:V012],
                            mybir.ActivationFunctionType.Copy,
                        )
                    else:
                        nc.vector.tensor_copy(vt[:], pss[i][:, 0:V012])
                    vts[st] = vt

        # ---- Attention-phase PSUM pools (4 + 2 + 2 = 8 banks) ----------
        def proj_steps(h, pool, copy_alt=False, tag="proj"):
            """Create head h's q/k tiles; return (qt, kt, generator) where
            the generator emits one 2-matmul chunk per next()."""
            qt = p_qt.tile([128, S], F16, tag="qt", name=f"qt{h}")
            kt = p_qt.tile([128, S], F16, tag="kt", name=f"kt{h}")

            def gen():
                n = 0
                order = [(qt, "wq", 0), (qt, "wq", 1), (kt, "wk", 0),
                         (kt, "wk", 1), (kt, "wk", 2), (kt, "wk", 3),
                         (qt, "wq", 2), (qt, "wq", 3)]
                for dst, wname, sb in order:
                    if True:
                        ps = pool.tile([128, 512], F32, tag=tag,
                                       name=f"pj{h}_{wname}{sb}")
                        for d in range(ND):
                            nc.tensor.matmul(
                                ps[:],
                                wsl(wname, d)[:, h * 128:(h + 1) * 128],
                                xts(d, sb),
                                start=(d == 0),
                                stop=(d == ND - 1),
                            )
                            if d % 2 == 1:
                                yield None
                        dsl = dst[:, sb * 512:(sb + 1) * 512]
                        if copy_alt and n % 2 == 0:
                            nc.scalar.activation(
                                dsl, ps[:], mybir.ActivationFunctionType.Copy
                            )
                        else:
                            nc.vector.tensor_copy(dsl, ps[:])
                        n += 1
                while True:
                    yield None

            return qt, kt, gen()

        # per-block attention state shared between qk/pv/norm emitters
        class Blk:
            def __init__(self, h, qb):
                self.h, self.qb = h, qb
                self.q0 = qb * 1024
                self.ets = {}
                self.acc = None
                self.pv = None
                self.final = False

        def qk_step(blk, qt, kt, k, pre=False):
            st_ps = ps_mm.tile([128, 1024], F32, tag="sT",
                               name=f"sT{blk.h}_{blk.qb}_{k}")
            for hf in range(2):
                nc.tensor.matmul(
                    st_ps[:, hf * 512:(hf + 1) * 512],
                    kt[:, k * 128:(k + 1) * 128],
                    qt[:, blk.q0 + hf * 512:blk.q0 + (hf + 1) * 512],
                    start=True,
                    stop=True,
                )
            et = p_exp.tile([128, 1024], F16, tag="pre" if pre else "exp",
                            bufs=14 if pre else 9, name=f"et{blk.h}_{blk.qb}_{k}")
            nc.scalar.activation(et[:], st_ps[:], mybir.ActivationFunctionType.Exp)
            blk.ets[k] = et
            # fold the row-sum accumulator as soon as exp lands, so the
            # denominator chain starts before the last PV
            if blk.final:
                # two parallel fold chains (even/odd k through 13) so no
                # serial DVE chain lags the tail; et14/et15 are summed by
                # the row-sum matmuls directly
                if k == 2:
                    acc = p_exp.tile([128, 1024], F16, tag="acc", bufs=5,
                                     name="acc_e")
                    nc.vector.tensor_add(acc[:], blk.ets[0][:], et[:])
                    blk.acc = acc
                elif k == 3:
                    acc = p_exp.tile([128, 1024], F16, tag="acc", bufs=5,
                                     name="acc_o")
                    nc.vector.tensor_add(acc[:], blk.ets[1][:], et[:])
                    blk.acc_o = acc
                elif 4 <= k <= 13:
                    dst = blk.acc if k % 2 == 0 else blk.acc_o
                    nc.vector.tensor_add(dst[:], dst[:], et[:])
            elif k == 1:
                acc = p_exp.tile([128, 1024], F16, tag="acc", bufs=5,
                                 name=f"acc{blk.h}_{blk.qb}")
                nc.vector.tensor_add(acc[:], blk.ets[0][:], et[:])
                blk.acc = acc
            elif k > 1:
                nc.vector.tensor_add(blk.acc[:], blk.acc[:], et[:])

        def pv_step(blk, k):
            if blk.final and k >= 14:
                et = blk.ets[k]         # norm_final still needs it
            else:
                et = blk.ets.pop(k)
            if blk.h < 3:
                vsl = vts[k][:, blk.h * 128:(blk.h + 1) * 128]
            else:
                vsl = vts3[k // 2][:, (k % 2) * 128:(k % 2 + 1) * 128]
            for hf in range(2):
                sl = slice(hf * 512, (hf + 1) * 512)
                nc.tensor.matmul(
                    blk.pv[:, sl],
                    vsl,
                    et[:, sl],
                    start=(k == 0),
                    stop=(k == KT - 1),
                )

        def norm_steps(blk, ps_pj):
            """Softmax-denominator chain for a non-final block; yields so the
            caller interleaves it with the next block's emission."""
            h, qb, acc, pv = blk.h, blk.qb, blk.acc, blk.pv
            # free the pv PSUM bank right away — the next block's first PV
            # matmul sits behind this chain in PE queue order
            ob = p_out.tile([128, 1024], F32, tag="o", name=f"ob{h}{qb}")
            nc.vector.tensor_copy(ob[:], pv[:])
            yield None
            sms = []
            for hf in range(2):
                sm = ps_pj.tile([1, 512], F32, tag="proj", name=f"sm{h}{qb}{hf}")
                nc.tensor.matmul(
                    sm[:], ones[:], acc[:, hf * 512:(hf + 1) * 512],
                    start=True, stop=True,
                )
                sms.append(sm)
            yield None
            sm_sb = p_rc.tile([1, 1024], F32, tag="sm_sb")
            for hf in range(2):
                nc.vector.tensor_copy(sm_sb[:, hf * 512:(hf + 1) * 512], sms[hf][:])
            sm2 = p_rc.tile([128, 8], F32, tag="sm2")
            nc.sync.dma_start(sm2[:], sm_sb[:], single_packet=True)
            rc2 = p_rc.tile([128, 8], F16, tag="rc2")
            with nc.allow_low_precision(reason="fp16 softmax denom"):
                nc.vector.reciprocal(rc2[:], sm2[:])
            r2dram = p_dram.tile([1, 1024], F16, tag="r2dram")
            nc.sync.dma_start(
                r2dram[:].rearrange("a (p c) -> (a p) c", p=128), rc2[:],
                single_packet=True,
            )
            rbc = p_rc.tile([128, 1024], F16, tag="rbc")
            nc.sync.dma_start(rbc[:], r2dram[0:1, :].to_broadcast((128, 1024)))
            yield None
            # halves, so a waiting multiply never blocks the DVE queue long
            obh = p_out.tile([128, 1024], F16, tag="oh", name=f"obh{h}{qb}")
            for hf in range(2):
                sl = slice(hf * 512, (hf + 1) * 512)
                nc.vector.tensor_mul(obh[:, sl], ob[:, sl], rbc[:, sl])
                nc.sync.dma_start(
                    out_ap[h * 128:(h + 1) * 128,
                           qb * 1024 + hf * 512:qb * 1024 + (hf + 1) * 512],
                    obh[:, sl],
                )
                yield None

        def norm_fast(blk, ps_pj):
            """DMA-less denominator chain for the second-to-last block: the
            3-DMA chain takes ~8us and would land mid-final-block, colliding
            with the tail; direct row-sum matmuls + PE transpose + selector
            broadcasts retire it early instead."""
            h, qb, acc, pv = blk.h, blk.qb, blk.acc, blk.pv
            ob = p_out.tile([128, 1024], F32, tag="o", name=f"obf{h}{qb}")
            nc.vector.tensor_copy(ob[:], pv[:])
            yield None
            smq = ps_pj.tile([128, 512], F32, tag="proj", name=f"smqf{h}{qb}")
            for j in range(8):
                nc.tensor.matmul(
                    smq[:, j:j + 1], acc[:, j * 128:(j + 1) * 128], ones[:],
                    start=True, stop=True, skip_group_check=True,
                )
            yield None
            rc2 = p_rc.tile([128, 8], F16, tag="rc2")
            with nc.allow_low_precision(reason="fp16 softmax denom"):
                nc.vector.reciprocal(rc2[:], smq[:, 0:8])
            yield None
            tps = ps_pj.tile([128, 512], F32, tag="proj", name=f"tpsf{h}{qb}")
            t16 = tps[:].bitcast(F16)
            nc.tensor.transpose(t16[0:8, 0:128], rc2[:], ident[:])
            t_sb = p_rc.tile([8, 128], F16, tag="t_sb")
            nc.vector.tensor_copy(t_sb[:], t16[0:8, 0:128])
            for hf in range(2):
                rbc = ps_pj.tile([128, 512], F32, tag="proj",
                                 name=f"rbcf{h}{qb}{hf}")
                for j in range(4):
                    jj = hf * 4 + j
                    nc.tensor.matmul(
                        rbc[:, j * 128:(j + 1) * 128],
                        sel8[:, jj * 128:(jj + 1) * 128], t_sb[:],
                        start=True, stop=True, skip_group_check=True,
                    )
                obh = p_out.tile([128, 512], F16, tag="of",
                                 name=f"obf2{h}{qb}{hf}")
                nc.vector.tensor_mul(
                    obh[:], ob[:, hf * 512:(hf + 1) * 512], rbc[:]
                )
                nc.sync.dma_start(
                    out_ap[h * 128:(h + 1) * 128,
                           qb * 1024 + hf * 512:qb * 1024 + (hf + 1) * 512],
                    obh[:],
                )
                yield None

        def norm_final(blk):
            """Tail chain for the very last block: row sums straight into
            [128, 8] via tiny stationary matmuls over the (k<=13) fold plus
            et14/et15 (PE is idle here and the fold chain lags ~2us), one
            unshuffle DMA, then a PE broadcast — minimizes serial DMAs."""
            h, qb, acc, pv = blk.h, blk.qb, blk.acc, blk.pv
            smq = ps_pj.tile([128, 512], F32, tag="proj", name="smq")
            srcs = [acc, blk.acc_o, blk.ets[14], blk.ets[15]]
            for j in range(8):
                # stationary column m is q = j*128 + m: smq[p, j] holds
                # rowsum(q = j*128 + p)
                for si, src in enumerate(srcs):
                    nc.tensor.matmul(
                        smq[:, j:j + 1], src[:, j * 128:(j + 1) * 128], ones[:],
                        start=(si == 0), stop=(si == len(srcs) - 1),
                        skip_group_check=True,
                    )
            rc2 = p_rc.tile([128, 8], F16, tag="rc2")
            with nc.allow_low_precision(reason="fp16 softmax denom"):
                nc.vector.reciprocal(rc2[:], smq[:, 0:8])
            # stay on-chip: PE transpose + per-block broadcast matmuls skip
            # the ~2.5us of DMA fixed costs an unshuffle round trip takes
            tps = ps_pj.tile([128, 512], F32, tag="proj", name="tps")
            t16 = tps[:].bitcast(F16)
            nc.tensor.transpose(t16[0:8, 0:128], rc2[:], ident[:])
            t_sb = p_rc.tile([8, 128], F16, tag="t_sb")
            nc.vector.tensor_copy(t_sb[:], t16[0:8, 0:128])
            # stage pv in SBUF (DVE may read only one PSUM input) on ACT,
            # which is idle in the tail — keeps DVE free for the recip chain
            ob_pv = p_out.tile([128, 1024], F32, tag="o", name="ob_pv")
            nc.scalar.activation(
                ob_pv[:, 0:512], pv[:, 0:512],
                mybir.ActivationFunctionType.Copy,
            )
            rbc_ps = ps_mm.tile([128, 1024], F32, tag="sT", name="rbc_ps")
            for j in range(8):
                # sel[:, j-block] is the row-j indicator: out = T[j, :] bcast
                nc.tensor.matmul(
                    rbc_ps[:, j * 128:(j + 1) * 128],
                    sel8[:, j * 128:(j + 1) * 128], t_sb[:],
                    start=True, stop=True, skip_group_check=True,
                )
            for hf in range(2):
                sl = slice(hf * 512, (hf + 1) * 512)
                if hf == 1:
                    nc.scalar.activation(
                        ob_pv[:, 512:1024], pv[:, 512:1024],
                        mybir.ActivationFunctionType.Copy,
                    )
                ob = p_out.tile([128, 512], F16, tag="of", name=f"of{hf}")
                nc.vector.tensor_mul(ob[:], ob_pv[:, sl], rbc_ps[:, sl])
                nc.sync.dma_start(
                    out_ap[h * 128:(h + 1) * 128,
                           qb * 1024 + hf * 512:qb * 1024 + (hf + 1) * 512],
                    ob[:],
                )

        # head 0's projections run serially (nothing to hide them under) and
        # share the phase-A PSUM pool, so they don't wait on its release;
        # heads 1..3 project inside earlier blocks' attention loops.
        q0_, k0_, gen = proj_steps(0, ps_v, copy_alt=True, tag="v")
        for _ in range(33):     # 8 chunks x 4 yields + 1: the final copy
            next(gen)           # is only emitted on the next() AFTER the
                                # last chunk's 4th yield
        ps_v.release()
        ps_mm = ctx.enter_context(tc.tile_pool(name="psmm", bufs=2, space="PSUM"))
        ps_pv = ctx.enter_context(tc.tile_pool(name="pspv", bufs=1, space="PSUM"))

        with tc.tile_pool(name="pspj", bufs=2, space="PSUM") as ps_pj:
            qts, kts = {0: q0_}, {0: k0_}
            projs = {}

            def make_proj(h):
                qth, kth, g = proj_steps(h, ps_pj)
                qts[h], kts[h], projs[h] = qth, kth, g

            make_proj(1)
            blocks = [Blk(b // 2, b % 2) for b in range(8)]
            blocks[7].final = True
            # per-block drip-feed plan:
            #   proj[b] = (head whose projection is injected, total pairs)
            #   pre[b]  = how many qk+exp steps of block b+1 to pre-execute
            # Each exp is 1038ns on ACT vs 852ns of matching qk+pv on PE, so
            # blocks with no injected work go ACT-bound. Shifting every
            # projection one block early lets each block pre-execute the
            # next one's first qk+exp steps (the cascade below), and head
            # 3's V projection fills block 5.
            # 33 next()s per generator: the final copy is emitted on the
            # call after the last chunk's 4th yield
            proj_plan = {0: (1, 33), 1: (2, 16), 2: (2, 17), 3: (3, 20), 4: (3, 13)}
            pre_plan = {1: 3, 2: 2, 3: 5, 4: 9, 5: 13, 6: 6}
            norm_in = None

            def emit_v3_chunk(sp):
                # block 5 has no projection in flight, so the proj ring is free
                ps = ps_pj.tile([128, 512], F32, tag="proj", name=f"v3_{sp}")
                for j in range(2):
                    st = sp * 2 + j
                    for d in range(ND):
                        nc.tensor.matmul(
                            ps[:, j * 128:(j + 1) * 128],
                            xts(d, st // 4)[:, (st % 4) * 128:(st % 4 + 1) * 128],
                            wsl("wv", d)[:, V012:DHG],
                            start=(d == 0),
                            stop=(d == ND - 1),
                            skip_group_check=True,
                        )
                vt = p_v.tile([128, 256], F16, tag="v3", bufs=8,
                              name=f"vt3_{sp}")
                nc.vector.tensor_copy(vt[:], ps[:, 0:256])
                vts3[sp] = vt

            for b, blk in enumerate(blocks):
                h = blk.h
                qt, kt = qts[h], kts[h]
                blk.pv = ps_pv.tile([128, 1024], F32, tag="pv",
                                    name=f"pv{h}_{blk.qb}")
                k0 = len(blk.ets)
                kq, kp, it = k0, 0, 0
                pre_left = pre_plan.get(b, 0)
                ph, pairs_left = proj_plan.get(b, (None, 0))
                if ph is not None and ph not in projs:
                    make_proj(ph)
                nproj = projs.get(ph)
                v3_left = 8 if b == 5 else 0

                if kq < KT:
                    qk_step(blk, qt, kt, kq)
                    kq += 1
                if norm_in is not None:
                    next(norm_in, None)         # pv-freeing copy
                if kq < KT:
                    qk_step(blk, qt, kt, kq)
                    kq += 1
                if norm_in is not None:
                    next(norm_in, None)         # row-sum matmuls
                if k0 > 0 and norm_in is not None:
                    next(norm_in, None)         # reciprocal DMA chain

                while kp < KT:
                    if k0 > 0:
                        # pre-filled block: pv leads so the exp-ring WAR
                        # order stays correct (pv(k) must be emitted before
                        # qk(k + ring) reuses et(k)'s slot)
                        pv_step(blk, kp)
                        kp += 1
                        if kq < KT:
                            qk_step(blk, qt, kt, kq)
                            kq += 1
                    elif kq < KT:
                        qk_step(blk, qt, kt, kq)
                        kq += 1
                        if kq - 2 >= kp:
                            pv_step(blk, kp)
                            kp += 1
                    else:
                        pv_step(blk, kp)
                        kp += 1
                    it += 1
                    if it == 1 and k0 == 0 and norm_in is not None:
                        next(norm_in, None)     # reciprocal DMA chain
                    if it == 5 and norm_in is not None:
                        # broadcast is in flight by now; the multiplies
                        # won't head-of-line-block the DVE queue for long
                        for _ in norm_in:
                            pass
                        norm_in = None
                    iters_left = max(1, 15 - it)
                    n_inj = min(pairs_left, 2,
                                (pairs_left + iters_left - 1) // iters_left)
                    for _ in range(n_inj):
                        next(nproj)
                        pairs_left -= 1
                    if pre_left > 0 and it >= 1:
                        nblk = blocks[b + 1]
                        pk = len(nblk.ets)
                        qk_step(nblk, qts[nblk.h], kts[nblk.h], pk, pre=True)
                        pre_left -= 1
                    if v3_left > 0 and it % 2 == 0:
                        emit_v3_chunk(8 - v3_left)
                        v3_left -= 1
                while pairs_left > 0:
                    next(nproj)
                    pairs_left -= 1
                while v3_left > 0:
                    emit_v3_chunk(8 - v3_left)
                    v3_left -= 1
                if b in (5, 6):
                    norm_in = norm_fast(blk, ps_pj)
                elif b < 7:
                    norm_in = norm_steps(blk, ps_pj)
                else:
                    if norm_in is not None:
                        for _ in norm_in:
                            pass
                        norm_in = None
                    norm_final(blk)


def _build():
    nc = bacc.Bacc(
        "TRN2",
        target_bir_lowering=False,
        debug=False,
        enable_asserts=False,
        num_devices=N_CORES,
    )
    xt_ap = nc.dram_tensor("xt", [D, S], F16, kind="ExternalInput").ap()
    wq_ap = nc.dram_tensor("wq", [128, ND * DHG], F16, kind="ExternalInput").ap()
    wk_ap = nc.dram_tensor("wk", [128, ND * DHG], F16, kind="ExternalInput").ap()
    wv_ap = nc.dram_tensor("wv", [128, ND * DHG], F16, kind="ExternalInput").ap()
    sel_ap = nc.dram_tensor("sel", [8, 1024], F16, kind="ExternalInput").ap()
    out_ap = nc.dram_tensor("out", [DHG, S], F16, kind="ExternalOutput").ap()
    with tile.TileContext(nc) as tc:
        _emit(tc, nc, xt_ap, wq_ap, wk_ap, wv_ap, sel_ap, out_ap)
    nc.compile()
    return nc


def _shard_inputs(inputs):
    x = np.ascontiguousarray(np.asarray(inputs["input_embeddings"], dtype=np.float32))
    wq = np.asarray(inputs["w_query"], dtype=np.float32) * SCALE
    wk = np.asarray(inputs["w_key"], dtype=np.float32)
    wv = np.asarray(inputs["w_value"], dtype=np.float32)

    def gather(w, g):
        # head h occupies the strided cols d = hd*8 + h; regroup head-major,
        # then d-major so each [128, 1024] DMA chunk is a plain slice
        w4 = w.reshape(D, DH, H)[:, :, g * HPC:(g + 1) * HPC]   # (D, hd, hl)
        wg = w4.transpose(0, 2, 1).reshape(ND, 128, DHG)
        return np.ascontiguousarray(
            wg.transpose(1, 0, 2).reshape(128, ND * DHG).astype(np.float16)
        )

    sel = np.kron(np.eye(8), np.ones((1, 128))).astype(np.float16)
    in_maps = []
    for c in range(N_CORES):
        b, g = divmod(c, 2)
        in_maps.append(
            {
                "xt": np.ascontiguousarray(x[b].T.astype(np.float16)),
                "wq": gather(wq, g),
                "wk": gather(wk, g),
                "wv": gather(wv, g),
                "sel": sel,
            }
        )
    return in_maps


def kernel(**inputs):
    nc = _CACHE.get("nc")
    if nc is None:
        nc = _CACHE["nc"] = _build()
    in_maps = _shard_inputs(inputs)
    res = run_bass_kernel_spmd(
        nc, in_maps, core_ids=list(range(N_CORES)), trace=TRACE
    )
    _CACHE["last_result"] = res
    out = np.empty((B, S, DH, H), dtype=np.float32)
    for c in range(N_CORES):
        b, g = divmod(c, 2)
        o = res.results[c]["out"].reshape(HPC, DH, S)            # (hl, hd, s)
        out[b, :, :, g * HPC:(g + 1) * HPC] = o.transpose(2, 1, 0)
    return out.reshape(B, S, D)


# revision 101
# speedup vs baseline: 1.0157x; 1.0006x over previous
"""Multi-head self-attention (B=4, S=2048, D=1024, H=8) on 8 TRN2 NeuronCores.

Sharding: core c -> batch b=c//2, head-group g=c%2 (4 heads/core).
Each core computes its 4 heads' attention output [512, 2048] (transposed,
head-major); the host gathers/reassembles the full [B, S, D] output.

Notes on the math: the reference adds the source mask per-QUERY (constant
along the key axis) before a softmax over keys, so the mask cancels exactly;
encoder_output_embedding and the target mask are unused by the reference.
The kernel therefore computes pure softmax(q k^T / sqrt(dh)) v, with the
1/sqrt(dh) scale folded into w_query on the host.

Schedule (per core):
  A) V = x @ wv, d-outer over 8 PSUM banks so the first matmul only waits
     on ~2 DMA chunks and the d-loop streams behind the DMA queue (the
     HWDGE processes one descriptor set per ~625ns, so inputs arrive as
     28 x 256KB chunks, not 56 x 128KB).
  B) head 0 q/k projection (PE-serial; nothing to hide it under).
  C) per head: flash-style attention with the next head's projection
     matmuls drip-fed into the ACT-paced inner loop. ACT (exp) has slack
     in heads 0-2 but is the binding engine in head 3, so head 2 hosts
     head 3's full projection in its first block and pre-executes the
     first 8 QK+exp steps of head 3's first block in its second; head 3's
     first block pre-executes 3 exp steps of the second. Row sums fold on
     DVE right after each exp; the softmax denominator pipeline overlaps
     the trailing PV matmuls and the next block's QK.
"""

import math
from contextlib import ExitStack

import numpy as np

import concourse.bacc as bacc
import concourse.tile as tile
from concourse import masks, mybir
from concourse.bass_utils import run_bass_kernel_spmd

N_CORES = 8
B, S, D, H = 4, 2048, 1024, 8
DH = 128                    # head dim
HPC = 4                     # heads per core
DHG = HPC * DH              # 512: projected width per core
SCALE = 1.0 / math.sqrt(DH)
KT = S // 128               # 16 key tiles
ND = D // 128               # 8 contraction tiles
NSB = S // 512              # 4 column blocks of x

F32 = mybir.dt.float32
F16 = mybir.dt.float16

TRACE = False               # test.py flips this for profiling runs
_CACHE = {}


def _emit(tc, nc, xt_ap, wq_ap, wk_ap, wv_ap, sel_ap, out_ap):
    with ExitStack() as ctx:
        p_xt = ctx.enter_context(tc.tile_pool(name="xt", bufs=16))
        p_w = ctx.enter_context(tc.tile_pool(name="w", bufs=4))
        # 3 live per tag: head h-1 still being read by its last block while
        # head h is read and head h+1 is being projected (plan shifts the
        # projections one block early)
        p_qt = ctx.enter_context(tc.tile_pool(name="qt", bufs=3))
        p_v = ctx.enter_context(tc.tile_pool(name="v", bufs=KT))
        p_exp = ctx.enter_context(tc.tile_pool(name="exp", bufs=6))
        p_out = ctx.enter_context(tc.tile_pool(name="o", bufs=2))
        p_rc = ctx.enter_context(tc.tile_pool(name="rc", bufs=2))
        p_const = ctx.enter_context(tc.tile_pool(name="const", bufs=1))
        p_dram = ctx.enter_context(tc.tile_pool(name="dram", bufs=2, space="DRAM"))

        ones = p_const.tile([128, 1], F16, tag="ones")
        nc.vector.memset(ones[:], 1.0)
        ones_row = p_const.tile([1, 128], F16, tag="ones_row")
        nc.vector.memset(ones_row[:], 1.0)
        ident = p_const.tile([128, 128], F16, tag="ident")
        masks.make_identity(nc, ident[:])
        sel8 = p_const.tile([8, 1024], F16, tag="sel8")

        # DMA chunking: [128, 1024] chunks (2KB/partition) halve the count
        # of 625ns HWDGE descriptor slots vs per-tile loads. xh[d][half]
        # covers x^T rows d*128.. cols half*1024..; wc[name][dp] packs two
        # 128-row weight chunks side by side.
        xh = [[None, None] for _ in range(ND)]
        wc = {"wv": [None] * 4, "wq": [None] * 4, "wk": [None] * 4}

        fine = {}                       # (kind, idx): [128, 512] head tiles

        def xts(d, sb):
            t = fine.get(("x", d, sb))
            if t is not None:
                return t[:]
            return xh[d][sb // 2][:, (sb % 2) * 512:(sb % 2) * 512 + 512]

        def wsl(name, d):
            t = fine.get((name, d))
            if t is not None:
                return t[:]
            return wc[name][d // 2][:, (d % 2) * DHG:(d % 2) * DHG + DHG]

        def dma_x(d, half):
            t = p_xt.tile([128, 1024], F16, tag="xt", name=f"x{d}_{half}")
            nc.sync.dma_start(
                t[:], xt_ap[d * 128:(d + 1) * 128, half * 1024:(half + 1) * 1024]
            )
            xh[d][half] = t

        def dma_w(name, ap, dp):
            # host lays weights out d-major: ap is [128, ND*DHG]
            t = p_w.tile([128, 2 * DHG], F16, tag=name, name=f"{name}{dp}")
            nc.sync.dma_start(t[:], ap[:, dp * 2 * DHG:(dp + 1) * 2 * DHG])
            wc[name][dp] = t

        # the first matmul needs only (wv[0], x[0, sb0]): issue those as
        # [128, 512] singles so PE starts ~1us sooner
        for d0 in range(2):
            t = p_w.tile([128, DHG], F16, tag="wvf", bufs=2, name=f"wvf{d0}")
            nc.sync.dma_start(t[:], wv_ap[:, d0 * DHG:(d0 + 1) * DHG])
            fine[("wv", d0)] = t
            t = p_xt.tile([128, 512], F16, tag="xtf", bufs=2, name=f"xf{d0}")
            nc.sync.dma_start(t[:], xt_ap[d0 * 128:(d0 + 1) * 128, 0:512])
            fine[("x", d0, 0)] = t
        for dp in range(4):
            if dp > 0:
                dma_w("wv", wv_ap, dp)
            dma_x(2 * dp, 0)
            dma_x(2 * dp + 1, 0)
        for dp in range(4):
            dma_w("wq", wq_ap, dp)
            dma_x(2 * dp, 1)
            dma_x(2 * dp + 1, 1)
        for dp in range(4):
            dma_w("wk", wk_ap, dp)
        nc.sync.dma_start(sel8[:], sel_ap[:, :])   # only needed at the tail

        # ---- Phase A: V(heads 0-2) = x @ wv[:, :384], d-outer over PSUM --
        # head 3's V columns are deferred into block 5 (head2-qb1), which
        # otherwise has no projection work to hide its ACT-bound exp loop.
        V012 = 3 * DH
        vts = [None] * KT
        vts3 = [None] * (KT // 2)       # head-3 V, [128, 256] per st-pair
        ps_v = tc.alloc_tile_pool(name="psv", bufs=8, space="PSUM")
        if True:
            for wave in range(2):
                pss = [
                    ps_v.tile([128, 512], F32, tag="v", name=f"psv{wave}_{i}")
                    for i in range(8)
                ]
                for d in range(ND):
                    for i in range(8):
                        st = wave * 8 + i
                        nc.tensor.matmul(
                            pss[i][:, 0:V012],
                            xts(d, st // 4)[:, (st % 4) * 128:(st % 4 + 1) * 128],
                            wsl("wv", d)[:, 0:V012],
                            start=(d == 0),
                            stop=(d == ND - 1),
                        )
                for i in range(8):
                    st = wave * 8 + i
                    vt = p_v.tile([128, V012], F16, tag="v", name=f"vt{st}")
                    # alternate engines so the copies drain in half the time
                    # and wave 2's first matmuls aren't blocked on bank reuse
                    if i % 2 == 0:
                        nc.scalar.activation(
                            vt[:], pss[i][:, 0:V012],
                            mybir.ActivationFunctionType.Copy,
                        )
                    else:
                        nc.vector.tensor_copy(vt[:], pss[i][:, 0:V012])
                    vts[st] = vt

        # ---- Attention-phase PSUM pools (4 + 2 + 2 = 8 banks) ----------
        def proj_steps(h, pool, copy_alt=False, tag="proj"):
            """Create head h's q/k tiles; return (qt, kt, generator) where
            the generator emits one 2-matmul chunk per next()."""
            qt = p_qt.tile([128, S], F16, tag="qt", name=f"qt{h}")
            kt = p_qt.tile([128, S], F16, tag="kt", name=f"kt{h}")

            def gen():
                n = 0
                order = [(qt, "wq", 0), (qt, "wq", 1), (kt, "wk", 0),
                         (kt, "wk", 1), (kt, "wk", 2), (kt, "wk", 3),
                         (qt, "wq", 2), (qt, "wq", 3)]
                for dst, wname, sb in order:
                    if True:
                        ps = pool.tile([128, 512], F32, tag=tag,
                                       name=f"pj{h}_{wname}{sb}")
                        for d in range(ND):
                            nc.tensor.matmul(
                                ps[:],
                                wsl(wname, d)[:, h * 128:(h + 1) * 128],
                                xts(d, sb),
                                start=(d == 0),
                                stop=(d == ND - 1),
                            )
                            if d % 2 == 1:
                                yield None
                        dsl = dst[:, sb * 512:(sb + 1) * 512]
                        if copy_alt and n % 2 == 0:
                            nc.scalar.activation(
                                dsl, ps[:], mybir.ActivationFunctionType.Copy
                            )
                        else:
                            nc.vector.tensor_copy(dsl, ps[:])
                        n += 1
                while True:
                    yield None

            return qt, kt, gen()

        # per-block attention state shared between qk/pv/norm emitters
        class Blk:
            def __init__(self, h, qb):
                self.h, self.qb = h, qb
                self.q0 = qb * 1024
                self.ets = {}
                self.acc = None
                self.pv = None
                self.final = False

        def qk_step(blk, qt, kt, k, pre=False):
            st_ps = ps_mm.tile([128, 1024], F32, tag="sT",
                               name=f"sT{blk.h}_{blk.qb}_{k}")
            for hf in range(2):
                nc.tensor.matmul(
                    st_ps[:, hf * 512:(hf + 1) * 512],
                    kt[:, k * 128:(k + 1) * 128],
                    qt[:, blk.q0 + hf * 512:blk.q0 + (hf + 1) * 512],
                    start=True,
                    stop=True,
                )
            et = p_exp.tile([128, 1024], F16, tag="pre" if pre else "exp",
                            bufs=14 if pre else 9, name=f"et{blk.h}_{blk.qb}_{k}")
            nc.scalar.activation(et[:], st_ps[:], mybir.ActivationFunctionType.Exp)
            blk.ets[k] = et
            # fold the row-sum accumulator as soon as exp lands, so the
            # denominator chain starts before the last PV
            if blk.final:
                # two parallel fold chains (even/odd k through 13) so no
                # serial DVE chain lags the tail; et14/et15 are summed by
                # the row-sum matmuls directly
                if k == 2:
                    acc = p_exp.tile([128, 1024], F16, tag="acc", bufs=5,
                                     name="acc_e")
                    nc.vector.tensor_add(acc[:], blk.ets[0][:], et[:])
                    blk.acc = acc
                elif k == 3:
                    acc = p_exp.tile([128, 1024], F16, tag="acc", bufs=5,
                                     name="acc_o")
                    nc.vector.tensor_add(acc[:], blk.ets[1][:], et[:])
                    blk.acc_o = acc
                elif 4 <= k <= 13:
                    dst = blk.acc if k % 2 == 0 else blk.acc_o
                    nc.vector.tensor_add(dst[:], dst[:], et[:])
            elif k == 1:
                acc = p_exp.tile([128, 1024], F16, tag="acc", bufs=5,
                                 name=f"acc{blk.h}_{blk.qb}")
                nc.vector.tensor_add(acc[:], blk.ets[0][:], et[:])
                blk.acc = acc
            elif k > 1:
                nc.vector.tensor_add(blk.acc[:], blk.acc[:], et[:])

        def pv_step(blk, k):
            if blk.final and k >= 14:
                et = blk.ets[k]         # norm_final still needs it
            else:
                et = blk.ets.pop(k)
            if blk.h < 3:
                vsl = vts[k][:, blk.h * 128:(blk.h + 1) * 128]
            else:
                vsl = vts3[k // 2][:, (k % 2) * 128:(k % 2 + 1) * 128]
            for hf in range(2):
                sl = slice(hf * 512, (hf + 1) * 512)
                nc.tensor.matmul(
                    blk.pv[:, sl],
                    vsl,
                    et[:, sl],
                    start=(k == 0),
                    stop=(k == KT - 1),
                )

        def norm_steps(blk, ps_pj):
            """Softmax-denominator chain for a non-final block; yields so the
            caller interleaves it with the next block's emission."""
            h, qb, acc, pv = blk.h, blk.qb, blk.acc, blk.pv
            # free the pv PSUM bank right away — the next block's first PV
            # matmul sits behind this chain in PE queue order
            ob = p_out.tile([128, 1024], F32, tag="o", name=f"ob{h}{qb}")
            nc.vector.tensor_copy(ob[:], pv[:])
            yield None
            sms = []
            for hf in range(2):
                sm = ps_pj.tile([1, 512], F32, tag="proj", name=f"sm{h}{qb}{hf}")
                nc.tensor.matmul(
                    sm[:], ones[:], acc[:, hf * 512:(hf + 1) * 512],
                    start=True, stop=True,
                )
                sms.append(sm)
            yield None
            sm_sb = p_rc.tile([1, 1024], F32, tag="sm_sb")
            for hf in range(2):
                nc.vector.tensor_copy(sm_sb[:, hf * 512:(hf + 1) * 512], sms[hf][:])
            sm2 = p_rc.tile([128, 8], F32, tag="sm2")
            nc.sync.dma_start(sm2[:], sm_sb[:], single_packet=True)
            rc2 = p_rc.tile([128, 8], F16, tag="rc2")
            with nc.allow_low_precision(reason="fp16 softmax denom"):
                nc.vector.reciprocal(rc2[:], sm2[:])
            r2dram = p_dram.tile([1, 1024], F16, tag="r2dram")
            nc.sync.dma_start(
                r2dram[:].rearrange("a (p c) -> (a p) c", p=128), rc2[:],
                single_packet=True,
            )
            rbc = p_rc.tile([128, 1024], F16, tag="rbc")
            nc.sync.dma_start(rbc[:], r2dram[0:1, :].to_broadcast((128, 1024)))
            yield None
            # halves, so a waiting multiply never blocks the DVE queue long
            obh = p_out.tile([128, 1024], F16, tag="oh", name=f"obh{h}{qb}")
            for hf in range(2):
                sl = slice(hf * 512, (hf + 1) * 512)
                nc.vector.tensor_mul(obh[:, sl], ob[:, sl], rbc[:, sl])
                nc.sync.dma_start(
                    out_ap[h * 128:(h + 1) * 128,
                           qb * 1024 + hf * 512:qb * 1024 + (hf + 1) * 512],
                    obh[:, sl],
                )
                yield None

        def norm_fast(blk, ps_pj):
            """DMA-less denominator chain for the second-to-last block: the
            3-DMA chain takes ~8us and would land mid-final-block, colliding
            with the tail; direct row-sum matmuls + PE transpose + selector
            broadcasts retire it early instead."""
            h, qb, acc, pv = blk.h, blk.qb, blk.acc, blk.pv
            ob = p_out.tile([128, 1024], F32, tag="o", name=f"obf{h}{qb}")
            nc.vector.tensor_copy(ob[:], pv[:])
            yield None
            smq = ps_pj.tile([128, 512], F32, tag="proj", name=f"smqf{h}{qb}")
            for j in range(8):
                nc.tensor.matmul(
                    smq[:, j:j + 1], acc[:, j * 128:(j + 1) * 128], ones[:],
                    start=True, stop=True, skip_group_check=True,
                )
            yield None
            rc2 = p_rc.tile([128, 8], F16, tag="rc2")
            with nc.allow_low_precision(reason="fp16 softmax denom"):
                nc.vector.reciprocal(rc2[:], smq[:, 0:8])
            yield None
            tps = ps_pj.tile([128, 512], F32, tag="proj", name=f"tpsf{h}{qb}")
            t16 = tps[:].bitcast(F16)
            nc.tensor.transpose(t16[0:8, 0:128], rc2[:], ident[:])
            t_sb = p_rc.tile([8, 128], F16, tag="t_sb")
            nc.vector.tensor_copy(t_sb[:], t16[0:8, 0:128])
            for hf in range(2):
                rbc = ps_pj.tile([128, 512], F32, tag="proj",
                                 name=f"rbcf{h}{qb}{hf}")
                for j in range(4):
                    jj = hf * 4 + j
                    nc.tensor.matmul(
                        rbc[:, j * 128:(j + 1) * 128],
                        sel8[:, jj * 128:(jj + 1) * 128], t_sb[:],
                        start=True, stop=True, skip_group_check=True,
                    )
                obh = p_out.tile([128, 512], F16, tag="of",
                                 name=f"obf2{h}{qb}{hf}")
                nc.vector.tensor_mul(
                    obh[:], ob[:, hf * 512:(hf + 1) * 512], rbc[:]
                )
                nc.sync.dma_start(
                    out_ap[h * 128:(h + 1) * 128,
                           qb * 1024 + hf * 512:qb * 1024 + (hf + 1) * 512],
                    obh[:],
                )
                yield None

        def norm_final(blk):
            """Tail chain for the very last block: row sums straight into
            [128, 8] via tiny stationary matmuls over the (k<=13) fold plus
            et14/et15 (PE is idle here and the fold chain lags ~2us), one
            unshuffle DMA, then a PE broadcast — minimizes serial DMAs."""
            h, qb, acc, pv = blk.h, blk.qb, blk.acc, blk.pv
            smq = ps_pj.tile([128, 512], F32, tag="proj", name="smq")
            srcs = [acc, blk.acc_o, blk.ets[14], blk.ets[15]]
            for j in range(8):
                # stationary column m is q = j*128 + m: smq[p, j] holds
                # rowsum(q = j*128 + p)
                for si, src in enumerate(srcs):
                    nc.tensor.matmul(
                        smq[:, j:j + 1], src[:, j * 128:(j + 1) * 128], ones[:],
                        start=(si == 0), stop=(si == len(srcs) - 1),
                        skip_group_check=True,
                    )
            rc2 = p_rc.tile([128, 8], F16, tag="rc2")
            with nc.allow_low_precision(reason="fp16 softmax denom"):
                nc.vector.reciprocal(rc2[:], smq[:, 0:8])
            # stay on-chip: PE transpose + per-block broadcast matmuls skip
            # the ~2.5us of DMA fixed costs an unshuffle round trip takes
            tps = ps_pj.tile([128, 512], F32, tag="proj", name="tps")
            t16 = tps[:].bitcast(F16)
            nc.tensor.transpose(t16[0:8, 0:128], rc2[:], ident[:])
            t_sb = p_rc.tile([8, 128], F16, tag="t_sb")
            nc.vector.tensor_copy(t_sb[:], t16[0:8, 0:128])
            # stage pv in SBUF (DVE may read only one PSUM input) on ACT,
            # which is idle in the tail — keeps DVE free for the recip chain
            ob_pv = p_out.tile([128, 1024], F32, tag="o", name="ob_pv")
            nc.scalar.activation(
                ob_pv[:, 0:512], pv[:, 0:512],
                mybir.ActivationFunctionType.Copy,
            )
            rbc_ps = ps_mm.tile([128, 1024], F32, tag="sT", name="rbc_ps")
            for j in range(8):
                # sel[:, j-block] is the row-j indicator: out = T[j, :] bcast
                nc.tensor.matmul(
                    rbc_ps[:, j * 128:(j + 1) * 128],
                    sel8[:, j * 128:(j + 1) * 128], t_sb[:],
                    start=True, stop=True, skip_group_check=True,
                )
            for hf in range(2):
                sl = slice(hf * 512, (hf + 1) * 512)
                if hf == 1:
                    nc.scalar.activation(
                        ob_pv[:, 512:1024], pv[:, 512:1024],
                        mybir.ActivationFunctionType.Copy,
                    )
                ob = p_out.tile([128, 512], F16, tag="of", name=f"of{hf}")
                nc.vector.tensor_mul(ob[:], ob_pv[:, sl], rbc_ps[:, sl])
                nc.sync.dma_start(
                    out_ap[h * 128:(h + 1) * 128,
                           qb * 1024 + hf * 512:qb * 1024 + (hf + 1) * 512],
                    ob[:],
                )

        # head 0's projections run serially (nothing to hide them under) and
        # share the phase-A PSUM pool, so they don't wait on its release;
        # heads 1..3 project inside earlier blocks' attention loops.
        q0_, k0_, gen = proj_steps(0, ps_v, copy_alt=True, tag="v")
        for _ in range(33):     # 8 chunks x 4 yields + 1: the final copy
            next(gen)           # is only emitted on the next() AFTER the
                                # last chunk's 4th yield
        ps_v.release()
        ps_mm = ctx.enter_context(tc.tile_pool(name="psmm", bufs=2, space="PSUM"))
        ps_pv = ctx.enter_context(tc.tile_pool(name="pspv", bufs=1, space="PSUM"))

        with tc.tile_pool(name="pspj", bufs=2, space="PSUM") as ps_pj:
            qts, kts = {0: q0_}, {0: k0_}
            projs = {}

            def make_proj(h):
                qth, kth, g = proj_steps(h, ps_pj)
                qts[h], kts[h], projs[h] = qth, kth, g

            make_proj(1)
            blocks = [Blk(b // 2, b % 2) for b in range(8)]
            blocks[7].final = True
            # per-block drip-feed plan:
            #   proj[b] = (head whose projection is injected, total pairs)
            #   pre[b]  = how many qk+exp steps of block b+1 to pre-execute
            # Each exp is 1038ns on ACT vs 852ns of matching qk+pv on PE, so
            # blocks with no injected work go ACT-bound. Shifting every
            # projection one block early lets each block pre-execute the
            # next one's first qk+exp steps (the cascade below), and head
            # 3's V projection fills block 5.
            # 33 next()s per generator: the final copy is emitted on the
            # call after the last chunk's 4th yield
            proj_plan = {0: (1, 33), 1: (2, 16), 2: (2, 17), 3: (3, 20), 4: (3, 13)}
            pre_plan = {1: 3, 2: 2, 3: 5, 4: 9, 5: 13, 6: 6}
            norm_in = None

            def emit_v3_chunk(sp):
                # block 5 has no projection in flight, so the proj ring is free
                ps = ps_pj.tile([128, 512], F32, tag="proj", name=f"v3_{sp}")
                for j in range(2):
                    st = sp * 2 + j
                    for d in range(ND):
                        nc.tensor.matmul(
                            ps[:, j * 128:(j + 1) * 128],
                            xts(d, st // 4)[:, (st % 4) * 128:(st % 4 + 1) * 128],
                            wsl("wv", d)[:, V012:DHG],
                            start=(d == 0),
                            stop=(d == ND - 1),
                            skip_group_check=True,
                        )
                vt = p_v.tile([128, 256], F16, tag="v3", bufs=8,
                              name=f"vt3_{sp}")
                nc.vector.tensor_copy(vt[:], ps[:, 0:256])
                vts3[sp] = vt

            for b, blk in enumerate(blocks):
                h = blk.h
                qt, kt = qts[h], kts[h]
                blk.pv = ps_pv.tile([128, 1024], F32, tag="pv",
                                    name=f"pv{h}_{blk.qb}")
                k0 = len(blk.ets)
                kq, kp, it = k0, 0, 0
                pre_left = pre_plan.get(b, 0)
                ph, pairs_left = proj_plan.get(b, (None, 0))
                if ph is not None and ph not in projs:
                    make_proj(ph)
                nproj = projs.get(ph)
                v3_left = 8 if b == 5 else 0

                if kq < KT:
                    qk_step(blk, qt, kt, kq)
                    kq += 1
                if norm_in is not None:
                    next(norm_in, None)         # pv-freeing copy
                if kq < KT:
                    qk_step(blk, qt, kt, kq)
                    kq += 1
                if norm_in is not None:
                    next(norm_in, None)         # row-sum matmuls
                elif b == 0:
                    for _ in range(3):          # front-load proj1 chunks
                        next(nproj)
                        pairs_left -= 1
                if k0 > 0 and norm_in is not None:
                    next(norm_in, None)         # reciprocal DMA chain

                while kp < KT:
                    if k0 > 0:
                        # pre-filled block: pv leads so the exp-ring WAR
                        # order stays correct (pv(k) must be emitted before
                        # qk(k + ring) reuses et(k)'s slot)
                        pv_step(blk, kp)
                        kp += 1
                        if kq < KT:
                            qk_step(blk, qt, kt, kq)
                            kq += 1
                    elif kq < KT:
                        qk_step(blk, qt, kt, kq)
                        kq += 1
                        if kq - 2 >= kp:
                            pv_step(blk, kp)
                            kp += 1
                    else:
                        pv_step(blk, kp)
                        kp += 1
                    it += 1
                    if it == 1 and k0 == 0 and norm_in is not None:
                        next(norm_in, None)     # reciprocal DMA chain
                    if it == 5 and norm_in is not None:
                        # broadcast is in flight by now; the multiplies
                        # won't head-of-line-block the DVE queue for long
                        for _ in norm_in:
                            pass
                        norm_in = None
                    iters_left = max(1, 15 - it)
                    n_inj = min(pairs_left, 2,
                                (pairs_left + iters_left - 1) // iters_left)
                    for _ in range(n_inj):
                        next(nproj)
                        pairs_left -= 1
                    if pre_left > 0 and it >= 1:
                        nblk = blocks[b + 1]
                        pk = len(nblk.ets)
                        qk_step(nblk, qts[nblk.h], kts[nblk.h], pk, pre=True)
                        pre_left -= 1
                    if v3_left > 0 and it % 2 == 0:
                        emit_v3_chunk(8 - v3_left)
                        v3_left -= 1
                while pairs_left > 0:
                    next(nproj)
                    pairs_left -= 1
                while v3_left > 0:
                    emit_v3_chunk(8 - v3_left)
                    v3_left -= 1
                if b in (5, 6):
                    norm_in = norm_fast(blk, ps_pj)
                elif b < 7:
                    norm_in = norm_steps(blk, ps_pj)
                else:
                    if norm_in is not None:
                        for _ in norm_in:
                            pass
                        norm_in = None
                    norm_final(blk)


def _build():
    nc = bacc.Bacc(
        "TRN2",
        target_bir_lowering=False,
        debug=False,
        enable_asserts=False,
        num_devices=N_CORES,
    )
    xt_ap = nc.dram_tensor("xt", [D, S], F16, kind="ExternalInput").ap()
    wq_ap = nc.dram_tensor("wq", [128, ND * DHG], F16, kind="ExternalInput").ap()
    wk_ap = nc.dram_tensor("wk", [128, ND * DHG], F16, kind="ExternalInput").ap()
    wv_ap = nc.dram_tensor("wv", [128, ND * DHG], F16, kind="ExternalInput").ap()
    sel_ap = nc.dram_tensor("sel", [8, 1024], F16, kind="ExternalInput").ap()
    out_ap = nc.dram_tensor("out", [DHG, S], F16, kind="ExternalOutput").ap()
    with tile.TileContext(nc) as tc:
        _emit(tc, nc, xt_ap, wq_ap, wk_ap, wv_ap, sel_ap, out_ap)
    nc.compile()
    return nc


def _shard_inputs(inputs):
    x = np.ascontiguousarray(np.asarray(inputs["input_embeddings"], dtype=np.float32))
    wq = np.asarray(inputs["w_query"], dtype=np.float32) * SCALE
    wk = np.asarray(inputs["w_key"], dtype=np.float32)
    wv = np.asarray(inputs["w_value"], dtype=np.float32)

    def gather(w, g):
        # head h occupies the strided cols d = hd*8 + h; regroup head-major,
        # then d-major so each [128, 1024] DMA chunk is a plain slice
        w4 = w.reshape(D, DH, H)[:, :, g * HPC:(g + 1) * HPC]   # (D, hd, hl)
        wg = w4.transpose(0, 2, 1).reshape(ND, 128, DHG)
        return np.ascontiguousarray(
            wg.transpose(1, 0, 2).reshape(128, ND * DHG).astype(np.float16)
        )

    sel = np.kron(np.eye(8), np.ones((1, 128))).astype(np.float16)
    in_maps = []
    for c in range(N_CORES):
        b, g = divmod(c, 2)
        in_maps.append(
            {
                "xt": np.ascontiguousarray(x[b].T.astype(np.float16)),
                "wq": gather(wq, g),
                "wk": gather(wk, g),
                "wv": gather(wv, g),
                "sel": sel,
            }
        )
    return in_maps


def kernel(**inputs):
    nc = _CACHE.get("nc")
    if nc is None:
        nc = _CACHE["nc"] = _build()
    in_maps = _shard_inputs(inputs)
    res = run_bass_kernel_spmd(
        nc, in_maps, core_ids=list(range(N_CORES)), trace=TRACE
    )
    _CACHE["last_result"] = res
    out = np.empty((B, S, DH, H), dtype=np.float32)
    for c in range(N_CORES):
        b, g = divmod(c, 2)
        o = res.results[c]["out"].reshape(HPC, DH, S)            # (hl, hd, s)
        out[b, :, :, g * HPC:(g + 1) * HPC] = o.transpose(2, 1, 0)
    return out.reshape(B, S, D)


# revision 104
# speedup vs baseline: 1.0166x; 1.0009x over previous
"""Multi-head self-attention (B=4, S=2048, D=1024, H=8) on 8 TRN2 NeuronCores.

Sharding: core c -> batch b=c//2, head-group g=c%2 (4 heads/core).
Each core computes its 4 heads' attention output [512, 2048] (transposed,
head-major); the host gathers/reassembles the full [B, S, D] output.

Notes on the math: the reference adds the source mask per-QUERY (constant
along the key axis) before a softmax over keys, so the mask cancels exactly;
encoder_output_embedding and the target mask are unused by the reference.
The kernel therefore computes pure softmax(q k^T / sqrt(dh)) v, with the
1/sqrt(dh) scale folded into w_query on the host.

Schedule (per core):
  A) V = x @ wv, d-outer over 8 PSUM banks so the first matmul only waits
     on ~2 DMA chunks and the d-loop streams behind the DMA queue (the
     HWDGE processes one descriptor set per ~625ns, so inputs arrive as
     28 x 256KB chunks, not 56 x 128KB).
  B) head 0 q/k projection (PE-serial; nothing to hide it under).
  C) per head: flash-style attention with the next head's projection
     matmuls drip-fed into the ACT-paced inner loop. ACT (exp) has slack
     in heads 0-2 but is the binding engine in head 3, so head 2 hosts
     head 3's full projection in its first block and pre-executes the
     first 8 QK+exp steps of head 3's first block in its second; head 3's
     first block pre-executes 3 exp steps of the second. Row sums fold on
     DVE right after each exp; the softmax denominator pipeline overlaps
     the trailing PV matmuls and the next block's QK.
"""

import math
from contextlib import ExitStack

import numpy as np

import concourse.bacc as bacc
import concourse.tile as tile
from concourse import masks, mybir
from concourse.bass_utils import run_bass_kernel_spmd

N_CORES = 8
B, S, D, H = 4, 2048, 1024, 8
DH = 128                    # head dim
HPC = 4                     # heads per core
DHG = HPC * DH              # 512: projected width per core
SCALE = 1.0 / math.sqrt(DH)
KT = S // 128               # 16 key tiles
ND = D // 128               # 8 contraction tiles
NSB = S // 512              # 4 column blocks of x

F32 = mybir.dt.float32
F16 = mybir.dt.float16

TRACE = False               # test.py flips this for profiling runs
_CACHE = {}


def _emit(tc, nc, xt_ap, wq_ap, wk_ap, wv_ap, sel_ap, out_ap):
    with ExitStack() as ctx:
        p_xt = ctx.enter_context(tc.tile_pool(name="xt", bufs=16))
        p_w = ctx.enter_context(tc.tile_pool(name="w", bufs=4))
        # 3 live per tag: head h-1 still being read by its last block while
        # head h is read and head h+1 is being projected (plan shifts the
        # projections one block early)
        p_qt = ctx.enter_context(tc.tile_pool(name="qt", bufs=3))
        p_v = ctx.enter_context(tc.tile_pool(name="v", bufs=KT))
        p_exp = ctx.enter_context(tc.tile_pool(name="exp", bufs=6))
        p_out = ctx.enter_context(tc.tile_pool(name="o", bufs=2))
        p_rc = ctx.enter_context(tc.tile_pool(name="rc", bufs=2))
        p_const = ctx.enter_context(tc.tile_pool(name="const", bufs=1))
        p_dram = ctx.enter_context(tc.tile_pool(name="dram", bufs=2, space="DRAM"))

        ones = p_const.tile([128, 1], F16, tag="ones")
        nc.vector.memset(ones[:], 1.0)
        ones_row = p_const.tile([1, 128], F16, tag="ones_row")
        nc.vector.memset(ones_row[:], 1.0)
        ident = p_const.tile([128, 128], F16, tag="ident")
        masks.make_identity(nc, ident[:])
        sel8 = p_const.tile([8, 1024], F16, tag="sel8")

        # DMA chunking: [128, 1024] chunks (2KB/partition) halve the count
        # of 625ns HWDGE descriptor slots vs per-tile loads. xh[d][half]
        # covers x^T rows d*128.. cols half*1024..; wc[name][dp] packs two
        # 128-row weight chunks side by side.
        xh = [[None, None] for _ in range(ND)]
        wc = {"wv": [None] * 4, "wq": [None] * 4, "wk": [None] * 4}

        fine = {}                       # (kind, idx): [128, 512] head tiles

        def xts(d, sb):
            t = fine.get(("x", d, sb))
            if t is not None:
                return t[:]
            return xh[d][sb // 2][:, (sb % 2) * 512:(sb % 2) * 512 + 512]

        def wsl(name, d):
            t = fine.get((name, d))
            if t is not None:
                return t[:]
            return wc[name][d // 2][:, (d % 2) * DHG:(d % 2) * DHG + DHG]

        def dma_x(d, half):
            t = p_xt.tile([128, 1024], F16, tag="xt", name=f"x{d}_{half}")
            nc.sync.dma_start(
                t[:], xt_ap[d * 128:(d + 1) * 128, half * 1024:(half + 1) * 1024]
            )
            xh[d][half] = t

        def dma_w(name, ap, dp):
            # host lays weights out d-major: ap is [128, ND*DHG]
            t = p_w.tile([128, 2 * DHG], F16, tag=name, name=f"{name}{dp}")
            nc.sync.dma_start(t[:], ap[:, dp * 2 * DHG:(dp + 1) * 2 * DHG])
            wc[name][dp] = t

        # the first matmul needs only (wv[0], x[0, sb0]): issue those as
        # [128, 512] singles so PE starts ~1us sooner
        for d0 in range(2):
            t = p_w.tile([128, DHG], F16, tag="wvf", bufs=2, name=f"wvf{d0}")
            nc.sync.dma_start(t[:], wv_ap[:, d0 * DHG:(d0 + 1) * DHG])
            fine[("wv", d0)] = t
            t = p_xt.tile([128, 512], F16, tag="xtf", bufs=2, name=f"xf{d0}")
            nc.sync.dma_start(t[:], xt_ap[d0 * 128:(d0 + 1) * 128, 0:512])
            fine[("x", d0, 0)] = t
        for dp in range(4):
            if dp > 0:
                dma_w("wv", wv_ap, dp)
            dma_x(2 * dp, 0)
            dma_x(2 * dp + 1, 0)
        for dp in range(4):
            dma_w("wq", wq_ap, dp)
            dma_x(2 * dp, 1)
            dma_x(2 * dp + 1, 1)
        for dp in range(4):
            dma_w("wk", wk_ap, dp)
        nc.sync.dma_start(sel8[:], sel_ap[:, :])   # only needed at the tail

        # ---- Phase A: V(heads 0-2) = x @ wv[:, :384], d-outer over PSUM --
        # head 3's V columns are deferred into block 5 (head2-qb1), which
        # otherwise has no projection work to hide its ACT-bound exp loop.
        V012 = 3 * DH
        vts = [None] * KT
        vts3 = [None] * (KT // 2)       # head-3 V, [128, 256] per st-pair
        ps_v = tc.alloc_tile_pool(name="psv", bufs=8, space="PSUM")
        if True:
            for wave in range(2):
                pss = [
                    ps_v.tile([128, 512], F32, tag="v", name=f"psv{wave}_{i}")
                    for i in range(8)
                ]
                for d in range(ND):
                    for i in range(8):
                        st = wave * 8 + i
                        nc.tensor.matmul(
                            pss[i][:, 0:V012],
                            xts(d, st // 4)[:, (st % 4) * 128:(st % 4 + 1) * 128],
                            wsl("wv", d)[:, 0:V012],
                            start=(d == 0),
                            stop=(d == ND - 1),
                        )
                for i in range(8):
                    st = wave * 8 + i
                    vt = p_v.tile([128, V012], F16, tag="v", name=f"vt{st}")
                    # alternate engines so the copies drain in half the time
                    # and wave 2's first matmuls aren't blocked on bank reuse
                    if i % 2 == 0:
                        nc.scalar.activation(
                            vt[:], pss[i][:, 0:V012],
                            mybir.ActivationFunctionType.Copy,
                        )
                    else:
                        nc.vector.tensor_copy(vt[:], pss[i][:, 0:V012])
                    vts[st] = vt

        # ---- Attention-phase PSUM pools (4 + 2 + 2 = 8 banks) ----------
        def proj_steps(h, pool, copy_alt=False, tag="proj"):
            """Create head h's q/k tiles; return (qt, kt, generator) where
            the generator emits one 2-matmul chunk per next()."""
            qt = p_qt.tile([128, S], F16, tag="qt", name=f"qt{h}")
            kt = p_qt.tile([128, S], F16, tag="kt", name=f"kt{h}")

            def gen():
                n = 0
                order = [(qt, "wq", 0), (qt, "wq", 1), (kt, "wk", 0),
                         (kt, "wk", 1), (kt, "wk", 2), (kt, "wk", 3),
                         (qt, "wq", 2), (qt, "wq", 3)]
                for dst, wname, sb in order:
                    if True:
                        ps = pool.tile([128, 512], F32, tag=tag,
                                       name=f"pj{h}_{wname}{sb}")
                        for d in range(ND):
                            nc.tensor.matmul(
                                ps[:],
                                wsl(wname, d)[:, h * 128:(h + 1) * 128],
                                xts(d, sb),
                                start=(d == 0),
                                stop=(d == ND - 1),
                            )
                            if d % 2 == 1:
                                yield None
                        dsl = dst[:, sb * 512:(sb + 1) * 512]
                        if copy_alt and n % 2 == 0:
                            nc.scalar.activation(
                                dsl, ps[:], mybir.ActivationFunctionType.Copy
                            )
                        else:
                            nc.vector.tensor_copy(dsl, ps[:])
                        n += 1
                while True:
                    yield None

            return qt, kt, gen()

        # per-block attention state shared between qk/pv/norm emitters
        class Blk:
            def __init__(self, h, qb):
                self.h, self.qb = h, qb
                self.q0 = qb * 1024
                self.ets = {}
                self.acc = None
                self.pv = None
                self.final = False

        def qk_step(blk, qt, kt, k, pre=False):
            st_ps = ps_mm.tile([128, 1024], F32, tag="sT",
                               name=f"sT{blk.h}_{blk.qb}_{k}")
            for hf in range(2):
                nc.tensor.matmul(
                    st_ps[:, hf * 512:(hf + 1) * 512],
                    kt[:, k * 128:(k + 1) * 128],
                    qt[:, blk.q0 + hf * 512:blk.q0 + (hf + 1) * 512],
                    start=True,
                    stop=True,
                )
            et = p_exp.tile([128, 1024], F16, tag="pre" if pre else "exp",
                            bufs=14 if pre else 9, name=f"et{blk.h}_{blk.qb}_{k}")
            nc.scalar.activation(et[:], st_ps[:], mybir.ActivationFunctionType.Exp)
            blk.ets[k] = et
            # fold the row-sum accumulator as soon as exp lands, so the
            # denominator chain starts before the last PV
            if blk.final:
                # two parallel fold chains (even/odd k through 13) so no
                # serial DVE chain lags the tail; et14/et15 are summed by
                # the row-sum matmuls directly
                if k == 2:
                    acc = p_exp.tile([128, 1024], F16, tag="acc", bufs=5,
                                     name="acc_e")
                    nc.vector.tensor_add(acc[:], blk.ets[0][:], et[:])
                    blk.acc = acc
                elif k == 3:
                    acc = p_exp.tile([128, 1024], F16, tag="acc", bufs=5,
                                     name="acc_o")
                    nc.vector.tensor_add(acc[:], blk.ets[1][:], et[:])
                    blk.acc_o = acc
                elif 4 <= k <= 13:
                    dst = blk.acc if k % 2 == 0 else blk.acc_o
                    nc.vector.tensor_add(dst[:], dst[:], et[:])
            elif k == 1:
                acc = p_exp.tile([128, 1024], F16, tag="acc", bufs=5,
                                 name=f"acc{blk.h}_{blk.qb}")
                nc.vector.tensor_add(acc[:], blk.ets[0][:], et[:])
                blk.acc = acc
            elif k > 1:
                nc.vector.tensor_add(blk.acc[:], blk.acc[:], et[:])

        def pv_step(blk, k):
            if blk.final and k >= 14:
                et = blk.ets[k]         # norm_final still needs it
            else:
                et = blk.ets.pop(k)
            if blk.h < 3:
                vsl = vts[k][:, blk.h * 128:(blk.h + 1) * 128]
            else:
                vsl = vts3[k // 2][:, (k % 2) * 128:(k % 2 + 1) * 128]
            for hf in range(2):
                sl = slice(hf * 512, (hf + 1) * 512)
                nc.tensor.matmul(
                    blk.pv[:, sl],
                    vsl,
                    et[:, sl],
                    start=(k == 0),
                    stop=(k == KT - 1),
                )

        def norm_steps(blk, ps_pj):
            """Softmax-denominator chain for a non-final block; yields so the
            caller interleaves it with the next block's emission."""
            h, qb, acc, pv = blk.h, blk.qb, blk.acc, blk.pv
            # free the pv PSUM bank right away — the next block's first PV
            # matmul sits behind this chain in PE queue order
            ob = p_out.tile([128, 1024], F32, tag="o", name=f"ob{h}{qb}")
            nc.vector.tensor_copy(ob[:], pv[:])
            yield None
            sms = []
            for hf in range(2):
                sm = ps_pj.tile([1, 512], F32, tag="proj", name=f"sm{h}{qb}{hf}")
                nc.tensor.matmul(
                    sm[:], ones[:], acc[:, hf * 512:(hf + 1) * 512],
                    start=True, stop=True,
                )
                sms.append(sm)
            yield None
            sm_sb = p_rc.tile([1, 1024], F32, tag="sm_sb")
            for hf in range(2):
                nc.vector.tensor_copy(sm_sb[:, hf * 512:(hf + 1) * 512], sms[hf][:])
            sm2 = p_rc.tile([128, 8], F32, tag="sm2")
            nc.sync.dma_start(sm2[:], sm_sb[:], single_packet=True)
            rc2 = p_rc.tile([128, 8], F16, tag="rc2")
            with nc.allow_low_precision(reason="fp16 softmax denom"):
                nc.vector.reciprocal(rc2[:], sm2[:])
            r2dram = p_dram.tile([1, 1024], F16, tag="r2dram")
            nc.sync.dma_start(
                r2dram[:].rearrange("a (p c) -> (a p) c", p=128), rc2[:],
                single_packet=True,
            )
            rbc = p_rc.tile([128, 1024], F16, tag="rbc")
            nc.sync.dma_start(rbc[:], r2dram[0:1, :].to_broadcast((128, 1024)))
            yield None
            # halves, so a waiting multiply never blocks the DVE queue long
            obh = p_out.tile([128, 1024], F16, tag="oh", name=f"obh{h}{qb}")
            for hf in range(2):
                sl = slice(hf * 512, (hf + 1) * 512)
                nc.vector.tensor_mul(obh[:, sl], ob[:, sl], rbc[:, sl])
                nc.sync.dma_start(
                    out_ap[h * 128:(h + 1) * 128,
                           qb * 1024 + hf * 512:qb * 1024 + (hf + 1) * 512],
                    obh[:, sl],
                )
                yield None

        def norm_fast(blk, ps_pj):
            """DMA-less denominator chain for the second-to-last block: the
            3-DMA chain takes ~8us and would land mid-final-block, colliding
            with the tail; direct row-sum matmuls + PE transpose + selector
            broadcasts retire it early instead."""
            h, qb, acc, pv = blk.h, blk.qb, blk.acc, blk.pv
            ob = p_out.tile([128, 1024], F32, tag="o", name=f"obf{h}{qb}")
            nc.vector.tensor_copy(ob[:], pv[:])
            yield None
            smq = ps_pj.tile([128, 512], F32, tag="proj", name=f"smqf{h}{qb}")
            for j in range(8):
                nc.tensor.matmul(
                    smq[:, j:j + 1], acc[:, j * 128:(j + 1) * 128], ones[:],
                    start=True, stop=True, skip_group_check=True,
                )
            yield None
            rc2 = p_rc.tile([128, 8], F16, tag="rc2")
            with nc.allow_low_precision(reason="fp16 softmax denom"):
                nc.vector.reciprocal(rc2[:], smq[:, 0:8])
            yield None
            tps = ps_pj.tile([128, 512], F32, tag="proj", name=f"tpsf{h}{qb}")
            t16 = tps[:].bitcast(F16)
            nc.tensor.transpose(t16[0:8, 0:128], rc2[:], ident[:])
            t_sb = p_rc.tile([8, 128], F16, tag="t_sb")
            nc.vector.tensor_copy(t_sb[:], t16[0:8, 0:128])
            for hf in range(2):
                rbc = ps_pj.tile([128, 512], F32, tag="proj",
                                 name=f"rbcf{h}{qb}{hf}")
                for j in range(4):
                    jj = hf * 4 + j
                    nc.tensor.matmul(
                        rbc[:, j * 128:(j + 1) * 128],
                        sel8[:, jj * 128:(jj + 1) * 128], t_sb[:],
                        start=True, stop=True, skip_group_check=True,
                    )
                obh = p_out.tile([128, 512], F16, tag="of",
                                 name=f"obf2{h}{qb}{hf}")
                nc.vector.tensor_mul(
                    obh[:], ob[:, hf * 512:(hf + 1) * 512], rbc[:]
                )
                nc.sync.dma_start(
                    out_ap[h * 128:(h + 1) * 128,
                           qb * 1024 + hf * 512:qb * 1024 + (hf + 1) * 512],
                    obh[:],
                )
                yield None

        def norm_final(blk):
            """Tail chain for the very last block: row sums straight into
            [128, 8] via tiny stationary matmuls over the (k<=13) fold plus
            et14/et15 (PE is idle here and the fold chain lags ~2us), one
            unshuffle DMA, then a PE broadcast — minimizes serial DMAs."""
            h, qb, acc, pv = blk.h, blk.qb, blk.acc, blk.pv
            smq = ps_pj.tile([128, 512], F32, tag="proj", name="smq")
            srcs = [acc, blk.acc_o, blk.ets[14], blk.ets[15]]
            for j in range(8):
                # stationary column m is q = j*128 + m: smq[p, j] holds
                # rowsum(q = j*128 + p)
                for si, src in enumerate(srcs):
                    nc.tensor.matmul(
                        smq[:, j:j + 1], src[:, j * 128:(j + 1) * 128], ones[:],
                        start=(si == 0), stop=(si == len(srcs) - 1),
                        skip_group_check=True,
                    )
            rc2 = p_rc.tile([128, 8], F16, tag="rc2")
            with nc.allow_low_precision(reason="fp16 softmax denom"):
                nc.vector.reciprocal(rc2[:], smq[:, 0:8])
            # stay on-chip: PE transpose + per-block broadcast matmuls skip
            # the ~2.5us of DMA fixed costs an unshuffle round trip takes
            tps = ps_pj.tile([128, 512], F32, tag="proj", name="tps")
            t16 = tps[:].bitcast(F16)
            nc.tensor.transpose(t16[0:8, 0:128], rc2[:], ident[:])
            t_sb = p_rc.tile([8, 128], F16, tag="t_sb")
            nc.vector.tensor_copy(t_sb[:], t16[0:8, 0:128])
            # stage pv in SBUF (DVE may read only one PSUM input) on ACT,
            # which is idle in the tail — keeps DVE free for the recip chain
            ob_pv = p_out.tile([128, 1024], F32, tag="o", name="ob_pv")
            nc.scalar.activation(
                ob_pv[:, 0:512], pv[:, 0:512],
                mybir.ActivationFunctionType.Copy,
            )
            rbc_ps = ps_mm.tile([128, 1024], F32, tag="sT", name="rbc_ps")
            for j in range(8):
                # sel[:, j-block] is the row-j indicator: out = T[j, :] bcast
                nc.tensor.matmul(
                    rbc_ps[:, j * 128:(j + 1) * 128],
                    sel8[:, j * 128:(j + 1) * 128], t_sb[:],
                    start=True, stop=True, skip_group_check=True,
                )
            for hf in range(2):
                sl = slice(hf * 512, (hf + 1) * 512)
                if hf == 1:
                    nc.scalar.activation(
                        ob_pv[:, 512:1024], pv[:, 512:1024],
                        mybir.ActivationFunctionType.Copy,
                    )
                ob = p_out.tile([128, 512], F16, tag="of", name=f"of{hf}")
                nc.vector.tensor_mul(ob[:], ob_pv[:, sl], rbc_ps[:, sl])
                nc.sync.dma_start(
                    out_ap[h * 128:(h + 1) * 128,
                           qb * 1024 + hf * 512:qb * 1024 + (hf + 1) * 512],
                    ob[:],
                )

        # head 0's projections run serially (nothing to hide them under) and
        # share the phase-A PSUM pool, so they don't wait on its release;
        # heads 1..3 project inside earlier blocks' attention loops.
        q0_, k0_, gen = proj_steps(0, ps_v, copy_alt=True, tag="v")
        for _ in range(33):     # 8 chunks x 4 yields + 1: the final copy
            next(gen)           # is only emitted on the next() AFTER the
                                # last chunk's 4th yield
        ps_v.release()
        ps_mm = ctx.enter_context(tc.tile_pool(name="psmm", bufs=2, space="PSUM"))
        ps_pv = ctx.enter_context(tc.tile_pool(name="pspv", bufs=1, space="PSUM"))

        with tc.tile_pool(name="pspj", bufs=2, space="PSUM") as ps_pj:
            qts, kts = {0: q0_}, {0: k0_}
            projs = {}

            def make_proj(h):
                qth, kth, g = proj_steps(h, ps_pj)
                qts[h], kts[h], projs[h] = qth, kth, g

            make_proj(1)
            blocks = [Blk(b // 2, b % 2) for b in range(8)]
            blocks[7].final = True
            # per-block drip-feed plan:
            #   proj[b] = (head whose projection is injected, total pairs)
            #   pre[b]  = how many qk+exp steps of block b+1 to pre-execute
            # Each exp is 1038ns on ACT vs 852ns of matching qk+pv on PE, so
            # blocks with no injected work go ACT-bound. Shifting every
            # projection one block early lets each block pre-execute the
            # next one's first qk+exp steps (the cascade below), and head
            # 3's V projection fills block 5.
            # 33 next()s per generator: the final copy is emitted on the
            # call after the last chunk's 4th yield
            proj_plan = {0: (1, 33), 1: (2, 16), 2: (2, 17), 3: (3, 20), 4: (3, 13)}
            pre_plan = {1: 3, 2: 2, 3: 5, 4: 9, 5: 13, 6: 6}
            norm_in = None

            def emit_v3_chunk(sp):
                # block 5 has no projection in flight, so the proj ring is free
                ps = ps_pj.tile([128, 512], F32, tag="proj", name=f"v3_{sp}")
                for j in range(2):
                    st = sp * 2 + j
                    for d in range(ND):
                        nc.tensor.matmul(
                            ps[:, j * 128:(j + 1) * 128],
                            xts(d, st // 4)[:, (st % 4) * 128:(st % 4 + 1) * 128],
                            wsl("wv", d)[:, V012:DHG],
                            start=(d == 0),
                            stop=(d == ND - 1),
                            skip_group_check=True,
                        )
                vt = p_v.tile([128, 256], F16, tag="v3", bufs=8,
                              name=f"vt3_{sp}")
                nc.vector.tensor_copy(vt[:], ps[:, 0:256])
                vts3[sp] = vt

            for b, blk in enumerate(blocks):
                h = blk.h
                qt, kt = qts[h], kts[h]
                blk.pv = ps_pv.tile([128, 1024], F32, tag="pv",
                                    name=f"pv{h}_{blk.qb}")
                k0 = len(blk.ets)
                kq, kp, it = k0, 0, 0
                pre_left = pre_plan.get(b, 0)
                ph, pairs_left = proj_plan.get(b, (None, 0))
                if ph is not None and ph not in projs:
                    make_proj(ph)
                nproj = projs.get(ph)
                v3_left = 8 if b == 5 else 0

                if kq < KT:
                    qk_step(blk, qt, kt, kq)
                    kq += 1
                if norm_in is not None:
                    next(norm_in, None)         # pv-freeing copy
                if kq < KT:
                    qk_step(blk, qt, kt, kq)
                    kq += 1
                if norm_in is not None:
                    next(norm_in, None)         # row-sum matmuls
                elif b == 0:
                    for _ in range(3):          # front-load proj1 chunks
                        next(nproj)
                        pairs_left -= 1
                if k0 > 0 and norm_in is not None:
                    next(norm_in, None)         # reciprocal DMA chain

                while kp < KT:
                    if k0 > 0:
                        # pre-filled block: pv leads so the exp-ring WAR
                        # order stays correct (pv(k) must be emitted before
                        # qk(k + ring) reuses et(k)'s slot)
                        pv_step(blk, kp)
                        kp += 1
                        if kq < KT:
                            qk_step(blk, qt, kt, kq)
                            kq += 1
                    elif kq < KT:
                        qk_step(blk, qt, kt, kq)
                        kq += 1
                        if kq - 2 >= kp:
                            pv_step(blk, kp)
                            kp += 1
                    else:
                        pv_step(blk, kp)
                        kp += 1
                    it += 1
                    if it == 1 and k0 == 0 and norm_in is not None:
                        next(norm_in, None)     # reciprocal DMA chain
                    if it == 6 and norm_in is not None:
                        # broadcast is in flight by now; the multiplies
                        # won't head-of-line-block the DVE queue for long
                        for _ in norm_in:
                            pass
                        norm_in = None
                    iters_left = max(1, 15 - it)
                    n_inj = min(pairs_left, 2,
                                (pairs_left + iters_left - 1) // iters_left)
                    for _ in range(n_inj):
                        next(nproj)
                        pairs_left -= 1
                    if pre_left > 0 and it >= 1:
                        nblk = blocks[b + 1]
                        pk = len(nblk.ets)
                        qk_step(nblk, qts[nblk.h], kts[nblk.h], pk, pre=True)
                        pre_left -= 1
                    if v3_left > 0 and it % 2 == 0:
                        emit_v3_chunk(8 - v3_left)
                        v3_left -= 1
                while pairs_left > 0:
                    next(nproj)
                    pairs_left -= 1
                while v3_left > 0:
                    emit_v3_chunk(8 - v3_left)
                    v3_left -= 1
                if b in (5, 6):
                    norm_in = norm_fast(blk, ps_pj)
                elif b < 7:
                    norm_in = norm_steps(blk, ps_pj)
                else:
                    if norm_in is not None:
                        for _ in norm_in:
                            pass
                        norm_in = None
                    norm_final(blk)


def _build():
    nc = bacc.Bacc(
        "TRN2",
        target_bir_lowering=False,
        debug=False,
        enable_asserts=False,
        num_devices=N_CORES,
    )
    xt_ap = nc.dram_tensor("xt", [D, S], F16, kind="ExternalInput").ap()
    wq_ap = nc.dram_tensor("wq", [128, ND * DHG], F16, kind="ExternalInput").ap()
    wk_ap = nc.dram_tensor("wk", [128, ND * DHG], F16, kind="ExternalInput").ap()
    wv_ap = nc.dram_tensor("wv", [128, ND * DHG], F16, kind="ExternalInput").ap()
    sel_ap = nc.dram_tensor("sel", [8, 1024], F16, kind="ExternalInput").ap()
    out_ap = nc.dram_tensor("out", [DHG, S], F16, kind="ExternalOutput").ap()
    with tile.TileContext(nc) as tc:
        _emit(tc, nc, xt_ap, wq_ap, wk_ap, wv_ap, sel_ap, out_ap)
    nc.compile()
    return nc


def _shard_inputs(inputs):
    x = np.ascontiguousarray(np.asarray(inputs["input_embeddings"], dtype=np.float32))
    wq = np.asarray(inputs["w_query"], dtype=np.float32) * SCALE
    wk = np.asarray(inputs["w_key"], dtype=np.float32)
    wv = np.asarray(inputs["w_value"], dtype=np.float32)

    def gather(w, g):
        # head h occupies the strided cols d = hd*8 + h; regroup head-major,
        # then d-major so each [128, 1024] DMA chunk is a plain slice
        w4 = w.reshape(D, DH, H)[:, :, g * HPC:(g + 1) * HPC]   # (D, hd, hl)
        wg = w4.transpose(0, 2, 1).reshape(ND, 128, DHG)
        return np.ascontiguousarray(
            wg.transpose(1, 0, 2).reshape(128, ND * DHG).astype(np.float16)
        )

    sel = np.kron(np.eye(8), np.ones((1, 128))).astype(np.float16)
    in_maps = []
    for c in range(N_CORES):
        b, g = divmod(c, 2)
        in_maps.append(
            {
                "xt": np.ascontiguousarray(x[b].T.astype(np.float16)),
                "wq": gather(wq, g),
                "wk": gather(wk, g),
                "wv": gather(wv, g),
                "sel": sel,
            }
        )
    return in_maps


def kernel(**inputs):
    nc = _CACHE.get("nc")
    if nc is None:
        nc = _CACHE["nc"] = _build()
    in_maps = _shard_inputs(inputs)
    res = run_bass_kernel_spmd(
        nc, in_maps, core_ids=list(range(N_CORES)), trace=TRACE
    )
    _CACHE["last_result"] = res
    out = np.empty((B, S, DH, H), dtype=np.float32)
    for c in range(N_CORES):
        b, g = divmod(c, 2)
        o = res.results[c]["out"].reshape(HPC, DH, S)            # (hl, hd, s)
        out[b, :, :, g * HPC:(g + 1) * HPC] = o.transpose(2, 1, 0)
    return out.reshape(B, S, D)
